# revision 32
# baseline (speedup 1.0000x reference)
"""Distributed GQA attention block (dense_transformer) for 8 TRN2 NeuronCores.

Reference computation (all fp32):
    q = (x @ wq)  -> RoPE;  k = (x @ wk) -> RoPE;  v = x @ wv
    causal softmax(q k^T / sqrt(64)) @ v  (GQA: 32 q heads, 4 kv heads)
    out = attn_out @ wo

Sharding: core (b, g) for b in {0,1}, g in {0..3} handles batch b, q-heads
8g..8g+7, kv-head g (data-parallel over batch x tensor-parallel over GQA
groups).  Each core computes attn_outT for its heads ([512, 2048],
feature-major), AllGathers within its 4-core batch group, and applies a
512-column slice of wo.  Outputs are disjoint -> host concat only.

All activations/weights are kept feature-major (transposed) on chip so every
matmul contracts over the partition dim with no on-chip transposes except a
single small one for v.  Matmul compute in bf16 (fp32 PSUM accumulate).
"""

import json

import numpy as np
import ml_dtypes

import concourse.bass as bass
import concourse.bass2jax as bass2jax
import concourse.mybir as mybir
import concourse.tile as tile
from concourse.tile import VectorClock, ScopedClock
from concourse.bass_utils import compile_bir_kernel, run_bass_kernel_spmd

_MAX_WAITS = 1  # this walrus build rejects instructions with more sem waits


def _split_excess_waits(bir_json, max_waits=_MAX_WAITS):
    """Hoist excess per-instruction sem waits onto injected same-engine NoOps.

    The TRN2 ISA encoding in this neuronxcc build allows at most `max_waits`
    sync-wait commands per instruction; Tile's sem assigner can emit more.
    A NoOp inserted immediately before the instruction on the same engine is
    semantically identical (the engine blocks at the same program point).
    """
    d = json.loads(bir_json)
    changed = False
    for fn in d.get("functions", []):
        for bb in fn.get("blocks", []):
            insts = bb.get("instructions", [])
            new = []
            for ins in insts:
                si = ins.get("sync_info")
                waits = (si or {}).get("on_wait") or []
                if len(waits) > max_waits:
                    changed = True
                    excess, keep = waits[:-max_waits], waits[-max_waits:]
                    for i in range(0, len(excess), max_waits):
                        new.append(
                            {
                                "debug": ins.get("debug", 0),
                                "engine": ins["engine"],
                                "ins": [],
                                "name": f"{ins['name']}-wsplit{i}",
                                "opcode": "NoOp",
                                "outs": [],
                                "sync_info": {
                                    "on_update": [],
                                    "on_wait": excess[i : i + max_waits],
                                },
                            }
                        )
                    si["on_wait"] = keep
                new.append(ins)
            bb["instructions"] = new
    if not changed:
        return bir_json
    return json.dumps(d).encode()


def _patched_compile_bir_kernel(bir_json, tmpdir, neff_name="file.neff"):
    return compile_bir_kernel(_split_excess_waits(bir_json), tmpdir, neff_name)


bass2jax.compile_bir_kernel = _patched_compile_bir_kernel

BF16 = ml_dtypes.bfloat16
F32 = mybir.dt.float32
BF = mybir.dt.bfloat16

DIM = 2048
T = 2048
HD = 64
N_CORES = 8
AF = mybir.ActivationFunctionType


class _TileContext(tile.TileContext):
    """TileContext whose final drain carries one sem wait per instruction.

    The walrus build in this image rejects a Drain carrying several sync
    waits ("Too many sync wait commands"), so emit individual single-wait
    NOPs on the sync engine first, then an unadorned drain + barriers.
    """

    def _drain_and_barrier(self, tick_clock, wait_clock):
        gc = tick_clock.global_clock
        vals = eval(repr(gc).replace("VectorClock(", "").rstrip(")"))
        for i, v in enumerate(vals):
            if v:
                single = [0] * len(vals)
                single[i] = v
                nop = self.nc.sync.nop(nofuse=True)
                wait_clock.add_sem_waits(
                    nop.ins, ScopedClock({None: VectorClock(single)})
                )
        self.nc.sync.drain()
        self.nc.all_engine_barrier()
        popped = self.nc._tile_sem_poison_stack.pop()
        assert popped is self._sem_poison
        self.nc.clear_and_free_semaphores(list(self.sems.allocated().values()))
        self.nc.all_engine_barrier()


def _build_nc():
    import os
    STAGE = int(os.environ.get("KSTAGE", "9"))
    nc = bass.Bass("TRN2")

    xt = nc.declare_dram_parameter("xt", [DIM, T], BF, isOutput=False)
    wq = nc.declare_dram_parameter("wq", [DIM, 512], BF, isOutput=False)
    wkv = nc.declare_dram_parameter("wkv", [DIM, 128], BF, isOutput=False)
    wo = nc.declare_dram_parameter("wo", [DIM, 512], BF, isOutput=False)
    cos2 = nc.declare_dram_parameter("cos2", [128, T], BF, isOutput=False)
    sin2 = nc.declare_dram_parameter("sin2", [128, T], BF, isOutput=False)
    coskv = nc.declare_dram_parameter("coskv", [128, T], BF, isOutput=False)
    sinkv = nc.declare_dram_parameter("sinkv", [128, T], BF, isOutput=False)
    r2t = nc.declare_dram_parameter("r2t", [128, 128], BF, isOutput=False)
    ident2 = nc.declare_dram_parameter("ident2", [128, 64], BF, isOutput=False)
    masks = nc.declare_dram_parameter("masks", [128, T], BF, isOutput=False)
    outt = nc.declare_dram_parameter("outt", [512, T], F32, isOutput=True)

    with _TileContext(nc) as tc:
        with (
            tc.tile_pool(name="consts", bufs=1) as consts,
            tc.tile_pool(name="big", bufs=1) as big,
            tc.tile_pool(name="wts", bufs=1) as wts,
            tc.tile_pool(name="acts", bufs=1) as acts,
            tc.tile_pool(name="work", bufs=4) as work,
            tc.tile_pool(name="exps", bufs=6) as exps,
            tc.tile_pool(name="outp", bufs=3) as outp,
            tc.tile_pool(name="psum", bufs=3, space="PSUM") as psum,
            tc.tile_pool(name="dram", bufs=1, space="DRAM") as dram,
        ):
            # ---- constants ----
            cos2_sb = consts.tile([128, T], BF)
            nc.sync.dma_start(cos2_sb[:], cos2[:])
            sin2_sb = consts.tile([128, T], BF)
            nc.sync.dma_start(sin2_sb[:], sin2[:])
            coskv_sb = consts.tile([128, T], BF)
            nc.sync.dma_start(coskv_sb[:], coskv[:])
            sinkv_sb = consts.tile([128, T], BF)
            nc.sync.dma_start(sinkv_sb[:], sinkv[:])
            masks_sb = consts.tile([128, T], BF)
            nc.sync.dma_start(masks_sb[:], masks[:])
            r2t_sb = consts.tile([128, 128], BF)
            nc.sync.dma_start(r2t_sb[:], r2t[:])
            ident2_sb = consts.tile([128, 64], BF)
            nc.sync.dma_start(ident2_sb[:], ident2[:])

            # ---- activations / weights in ----
            xt_sb = big.tile([128, 16, T], BF, tag="big")
            for fc in range(16):
                nc.sync.dma_start(xt_sb[:, fc, :], xt[fc * 128 : fc * 128 + 128, :])
            wkv_sb = wts.tile([128, 16, 128], BF)
            for fc in range(16):
                nc.sync.dma_start(
                    wkv_sb[:, fc, :], wkv[fc * 128 : fc * 128 + 128, :]
                )
            wq_sb = wts.tile([128, 16, 4, 128], BF)
            for fc in range(16):
                nc.sync.dma_start(
                    wq_sb[:, fc, :, :],
                    wq[fc * 128 : fc * 128 + 128, :].rearrange(
                        "p (qc m) -> p qc m", m=128
                    ),
                )
            wo_sb = wts.tile([128, 16, 4, 128], BF)

            # ---- kv projection + rope (k rows 0..63, v rows 64..127) ----
            kvrope_sb = acts.tile([128, T], BF)
            for tt in range(4 if STAGE >= 2 else 0):
                ts = slice(tt * 512, tt * 512 + 512)
                ps = psum.tile([128, 512], F32, tag="mm")
                for fc in range(16):
                    nc.tensor.matmul(
                        ps[:],
                        lhsT=wkv_sb[:, fc, :],
                        rhs=xt_sb[:, fc, ts],
                        start=(fc == 0),
                        stop=(fc == 15),
                    )
                kv_sb = work.tile([128, 512], BF, tag="evac")
                nc.vector.tensor_copy(kv_sb[:], ps[:])
                psu = psum.tile([128, 512], F32, tag="mm", name="psu")
                nc.tensor.matmul(
                    psu[:], lhsT=r2t_sb[:], rhs=kv_sb[:], start=True, stop=True
                )
                t1 = work.tile([128, 512], BF, tag="t1")
                nc.vector.tensor_mul(t1[:], kv_sb[:], coskv_sb[:, ts])
                t2 = work.tile([128, 512], BF, tag="t2")
                nc.vector.tensor_mul(t2[:], psu[:], sinkv_sb[:, ts])
                nc.vector.tensor_add(kvrope_sb[:, ts], t1[:], t2[:])

            # duplicate roped k into both partition halves (row-group packing)
            kdup_sb = acts.tile([128, T], BF)
            if STAGE >= 2:
                nc.sync.dma_start(kdup_sb[0:64, :], kvrope_sb[0:64, :])
                nc.sync.dma_start(kdup_sb[64:128, :], kvrope_sb[0:64, :])

            # v' chunks [128 tok, 65]: col 64 = 1.0 (softmax denominator trick)
            v1_sb = acts.tile([128, 16, 65], BF)
            nc.vector.memset(v1_sb[:, :, 64:65], 1.0)
            for kt in range(16 if STAGE >= 2 else 0):
                pst = psum.tile([128, 64], BF, tag="pav", bufs=2)
                nc.tensor.transpose(
                    pst[:],
                    kvrope_sb[64:128, kt * 128 : kt * 128 + 128],
                    ident2_sb[64:128, :],
                )
                nc.scalar.copy(v1_sb[:, kt, 0:64], pst[:])

            # ---- q projection chunks interleaved with attention head pairs ----
            qrope_sb = acts.tile([128, 4, T], BF)
            ao_q = [
                dram.tile([128, T], BF, name=f"aoq{i}") for i in range(4)
            ]
            aof_q = [
                dram.tile([512, T], BF, name=f"aofq{i}") for i in range(4)
            ]
            scale = 1.0 / np.sqrt(HD)
            aof_sb = big.tile([128, 16, T], BF, tag="big")

            for ph in range(4):
                if ph == 2 and STAGE >= 7:
                    # stream wo weights during attention (they are needed
                    # right after the last gather)
                    for fc in range(16):
                        nc.sync.dma_start(
                            wo_sb[:, fc, :, :],
                            wo[fc * 128 : fc * 128 + 128, :].rearrange(
                                "p (cc m) -> p cc m", m=128
                            ),
                        )
                if ph == 3 and STAGE >= 6:
                    # reload already-gathered quarters while ph3 computes
                    # (gpsimd queue is idle; these wait only for xt release)
                    for i in range(3):
                        for c in range(4):
                            nc.gpsimd.dma_start(
                                aof_sb[:, 4 * i + c, :],
                                aof_q[i][c * 128 : c * 128 + 128, :],
                            )
                if STAGE >= 3:
                  for tt in range(4):
                    ts = slice(tt * 512, tt * 512 + 512)
                    ps = psum.tile([128, 512], F32, tag="mm", name="psq")
                    for fc in range(16):
                        nc.tensor.matmul(
                            ps[:],
                            lhsT=wq_sb[:, fc, ph, :],
                            rhs=xt_sb[:, fc, ts],
                            start=(fc == 0),
                            stop=(fc == 15),
                        )
                    q_sb = work.tile([128, 512], BF, tag="evac")
                    nc.vector.tensor_copy(q_sb[:], ps[:])
                    psu = psum.tile([128, 512], F32, tag="mm", name="psu2")
                    nc.tensor.matmul(
                        psu[:], lhsT=r2t_sb[:], rhs=q_sb[:],
                        start=True, stop=True,
                    )
                    t1 = work.tile([128, 512], BF, tag="t1")
                    nc.vector.tensor_mul(t1[:], q_sb[:], cos2_sb[:, ts])
                    t2 = work.tile([128, 512], BF, tag="t2")
                    nc.vector.tensor_mul(t2[:], psu[:], sin2_sb[:, ts])
                    nc.vector.tensor_add(qrope_sb[:, ph, ts], t1[:], t2[:])

                for qb in range(4):
                    # attention for heads (2*ph, 2*ph+1), query tile qb
                    if STAGE < 4:
                        continue
                    Q0 = qb * 512
                    nkt = 4 * qb + 4
                    pav = [
                        psum.tile([65, 512], F32, tag="pav", name=f"pav{i}", bufs=2)
                        for i in range(2)
                    ]
                    for pr in range(nkt // 2):
                        kt0, kt1 = 2 * pr, 2 * pr + 1
                        # causal-active widths (tiles above the diagonal shrink)
                        j0, j1 = kt0 - 4 * qb, kt1 - 4 * qb
                        w0 = 512 if j0 < 0 else 512 - 128 * j0
                        w1 = 512 if j1 < 0 else 512 - 128 * j1
                        diag = j0 >= 0
                        # scores for both head halves interleaved so adjacent
                        # matmuls target different PE row groups (concurrent)
                        pss = [
                            psum.tile([128, 1024], F32, tag="mm", name=f"pss{i}")
                            for i in range(2)
                        ]
                        for kt, w, off in ((kt0, w0, 0), (kt1, w1, w0)):
                            for par in range(2):
                                lo, hi = (0, 64) if par == 0 else (64, 128)
                                nc.tensor.matmul(
                                    pss[par][:, off : off + w],
                                    lhsT=kdup_sb[lo:hi, kt * 128 : kt * 128 + 128],
                                    rhs=qrope_sb[lo:hi, ph, Q0 + 512 - w : Q0 + 512],
                                    start=True,
                                    stop=True,
                                )
                        e_pair = []
                        for par in range(2):
                            e_sb = exps.tile([128, 1024], BF, tag="e", name=f"e{par}")
                            nc.scalar.activation(
                                e_sb[:, 0 : w0 + w1], pss[par][:, 0 : w0 + w1],
                                AF.Exp, scale=scale,
                            )
                            if diag:
                                nc.vector.tensor_mul(
                                    e_sb[:, 0:w0], e_sb[:, 0:w0],
                                    masks_sb[:, 0:w0],
                                )
                                nc.vector.tensor_mul(
                                    e_sb[:, w0 : w0 + w1], e_sb[:, w0 : w0 + w1],
                                    masks_sb[:, 0:w1],
                                )
                            e_pair.append(e_sb)
                        for kt, w, off in ((kt0, w0, 0), (kt1, w1, w0)):
                            for par in range(2):
                                nc.tensor.matmul(
                                    pav[par][:, 512 - w : 512],
                                    lhsT=v1_sb[:, kt, :],
                                    rhs=e_pair[par][:, off : off + w],
                                    start=(kt == 0),
                                    stop=(kt == nkt - 1),
                                )
                    # evacuate unnormalized av + denominators (one copy per
                    # half), releasing the PSUM accumulators immediately; the
                    # normalization below runs off the critical path with no
                    # PE/PSUM involvement (DRAM-bounce broadcast), and both
                    # halves share one 128-lane reciprocal
                    avu = []
                    for par in range(2):
                        avu_sb = work.tile([65, 512], BF, tag="avu", name=f"avu{par}")
                        nc.scalar.copy(avu_sb[:], pav[par][:])
                        avu.append(avu_sb)
                    dden = dram.tile([2, 512], BF, tag="dden", bufs=4, name="dden")
                    for par in range(2):
                        nc.sync.dma_start(dden[par : par + 1, :], avu[par][64:65, :])
                    rden_sb = work.tile([128, 8], BF, tag="rden")
                    nc.sync.dma_start(
                        rden_sb[:],
                        bass.AP(tensor=dden.tensor, offset=dden.offset,
                                ap=[[8, 128], [1, 8]]),
                    )
                    with nc.allow_low_precision(
                        reason="bf16 softmax denominators are within tolerance"
                    ):
                        nc.vector.reciprocal(rden_sb[:], rden_sb[:])
                    rdden = dram.tile([2, 512], BF, tag="rdden", bufs=4, name="rdden")
                    nc.sync.dma_start(
                        bass.AP(tensor=rdden.tensor, offset=rdden.offset,
                                ap=[[8, 128], [1, 8]]),
                        rden_sb[:],
                    )
                    for par in range(2):
                        b_sb = work.tile([64, 512], BF, tag="bcast", name=f"b{par}")
                        nc.sync.dma_start(
                            b_sb[:],
                            bass.AP(
                                tensor=rdden.tensor,
                                offset=rdden[par : par + 1, :].offset,
                                ap=[[0, 64], [1, 512]],
                            ),
                        )
                        av_sb = work.tile([64, 512], BF, tag="av", name=f"av{par}")
                        nc.vector.tensor_mul(
                            av_sb[:], avu[par][0:64, :], b_sb[:]
                        )
                        nc.sync.dma_start(
                            ao_q[ph][64 * par : 64 * par + 64, Q0 : Q0 + 512],
                            av_sb[:],
                        )
                if STAGE >= 5:
                    # gather this head pair while later pairs compute
                    nc.gpsimd.collective_compute(
                        "AllGather",
                        mybir.AluOpType.bypass,
                        ins=[ao_q[ph].opt()],
                        outs=[aof_q[ph].opt()],
                        replica_groups=[[0, 1, 2, 3], [4, 5, 6, 7]],
                    )
                    if STAGE >= 6 and ph == 3:
                        for c in range(4):
                            nc.sync.dma_start(
                                aof_sb[:, 12 + c, :],
                                aof_q[3][c * 128 : c * 128 + 128, :],
                            )

            # ---- output projection (512-column slice of wo) ----
            # aof_q[i] chunk c covers rank c, head pair i of that rank
            #   -> wo feature-chunk 4*c + i
            NWO = int(os.environ.get('KWO', '4'))
            for cc in range(NWO if STAGE >= 7 else 0):
                for tt in range(4):
                    ts = slice(tt * 512, tt * 512 + 512)
                    ps = psum.tile([128, 512], F32, tag="mm")
                    for i in range(4):
                        for c in range(4):
                            nc.tensor.matmul(
                                ps[:],
                                lhsT=wo_sb[:, 4 * c + i, cc, :],
                                rhs=aof_sb[:, 4 * i + c, ts],
                                start=(i == 0 and c == 0),
                                stop=(i == 3 and c == 3),
                            )
                    o_sb = outp.tile([128, 512], F32, tag="o")
                    nc.scalar.copy(o_sb[:], ps[:])
                    nc.sync.dma_start(outt[cc * 128 : cc * 128 + 128, ts], o_sb[:])

    return nc


def _host_tables():
    inv_freq = 1.0 / (10000.0 ** (np.arange(0, HD, 2, dtype=np.float32) / HD))
    t = np.arange(T, dtype=np.float32)
    freqs = np.einsum("i,j->ij", t, inv_freq)
    emb = np.concatenate([freqs, freqs], axis=-1)  # [T, 64]
    cosT = np.cos(emb).T.astype(np.float32)  # [64, T]
    sinT = np.sin(emb).T.astype(np.float32)

    cos2 = np.ascontiguousarray(np.vstack([cosT, cosT])).astype(BF16)
    sin2 = np.ascontiguousarray(np.vstack([sinT, sinT])).astype(BF16)
    coskv = np.ascontiguousarray(np.vstack([cosT, np.ones_like(cosT)])).astype(BF16)
    sinkv = np.ascontiguousarray(np.vstack([sinT, np.zeros_like(sinT)])).astype(BF16)

    R = np.zeros((HD, HD), dtype=np.float32)
    for d in range(32):
        R[d, d + 32] = -1.0
        R[d + 32, d] = 1.0
    r2 = np.block([[R, np.zeros_like(R)], [np.zeros_like(R), R]])
    r2t = np.ascontiguousarray(r2.T).astype(BF16)  # lhsT: matmul computes R2 @ rhs

    ident2 = np.vstack([np.eye(HD), np.eye(HD)]).astype(BF16)  # [128, 64]

    masks = np.zeros((128, T), dtype=np.float32)
    r_idx = np.arange(128)[:, None]
    c_idx = np.arange(512)[None, :]
    for j in range(4):
        masks[:, j * 512 : j * 512 + 512] = (c_idx >= 128 * j + r_idx)
    masks = masks.astype(BF16)

    return dict(
        cos2=cos2, sin2=sin2, coskv=coskv, sinkv=sinkv,
        r2t=r2t, ident2=ident2, masks=masks,
    )


_STATE = {}


def _get_nc():
    if "nc" not in _STATE:
        _STATE["nc"] = _build_nc()
        _STATE["tables"] = _host_tables()
    return _STATE["nc"], _STATE["tables"]


def kernel(x, wq, wk, wv, wo):
    nc, tables = _get_nc()

    x = np.asarray(x, dtype=np.float32)
    wq_b = np.asarray(wq, dtype=np.float32).astype(BF16)
    wo_b = np.asarray(wo, dtype=np.float32).astype(BF16)
    wk_b = np.asarray(wk, dtype=np.float32).astype(BF16)
    wv_b = np.asarray(wv, dtype=np.float32).astype(BF16)

    in_maps = []
    xt_b = [np.ascontiguousarray(x[b].T).astype(BF16) for b in range(2)]
    for core in range(N_CORES):
        b, g = core // 4, core % 4
        m = dict(tables)
        m["xt"] = xt_b[b]
        m["wq"] = np.ascontiguousarray(wq_b[:, 512 * g : 512 * g + 512])
        m["wkv"] = np.ascontiguousarray(
            np.concatenate(
                [wk_b[:, 64 * g : 64 * g + 64], wv_b[:, 64 * g : 64 * g + 64]],
                axis=1,
            )
        )
        m["wo"] = np.ascontiguousarray(wo_b[:, 512 * g : 512 * g + 512])
        in_maps.append(m)

    res = run_bass_kernel_spmd(
        nc, in_maps, core_ids=list(range(N_CORES)), trace=False
    )

    out = np.empty((2, T, DIM), dtype=np.float32)
    for core in range(N_CORES):
        b, g = core // 4, core % 4
        out[b][:, 512 * g : 512 * g + 512] = res.results[core]["outt"].T
    return out


# revision 33
# speedup vs baseline: 1.0962x; 1.0962x over previous
"""Distributed GQA attention block (dense_transformer) for 8 TRN2 NeuronCores.

Reference computation (all fp32):
    q = (x @ wq)  -> RoPE;  k = (x @ wk) -> RoPE;  v = x @ wv
    causal softmax(q k^T / sqrt(64)) @ v  (GQA: 32 q heads, 4 kv heads)
    out = attn_out @ wo

Sharding: core (b, g) for b in {0,1}, g in {0..3} handles batch b, q-heads
8g..8g+7, kv-head g (data-parallel over batch x tensor-parallel over GQA
groups).  Each core computes attn_outT for its heads ([512, 2048],
feature-major), AllGathers within its 4-core batch group, and applies a
512-column slice of wo.  Outputs are disjoint -> host concat only.

All activations/weights are kept feature-major (transposed) on chip so every
matmul contracts over the partition dim with no on-chip transposes except a
single small one for v.  Matmul compute in bf16 (fp32 PSUM accumulate).
"""

import json

import numpy as np
import ml_dtypes

import concourse.bass as bass
import concourse.bass2jax as bass2jax
import concourse.mybir as mybir
import concourse.tile as tile
from concourse.tile import VectorClock, ScopedClock
from concourse.bass_utils import compile_bir_kernel, run_bass_kernel_spmd

_MAX_WAITS = 1  # this walrus build rejects instructions with more sem waits


def _split_excess_waits(bir_json, max_waits=_MAX_WAITS):
    """Hoist excess per-instruction sem waits onto injected same-engine NoOps.

    The TRN2 ISA encoding in this neuronxcc build allows at most `max_waits`
    sync-wait commands per instruction; Tile's sem assigner can emit more.
    A NoOp inserted immediately before the instruction on the same engine is
    semantically identical (the engine blocks at the same program point).
    """
    d = json.loads(bir_json)
    changed = False
    for fn in d.get("functions", []):
        for bb in fn.get("blocks", []):
            insts = bb.get("instructions", [])
            new = []
            for ins in insts:
                si = ins.get("sync_info")
                waits = (si or {}).get("on_wait") or []
                if len(waits) > max_waits:
                    changed = True
                    excess, keep = waits[:-max_waits], waits[-max_waits:]
                    for i in range(0, len(excess), max_waits):
                        new.append(
                            {
                                "debug": ins.get("debug", 0),
                                "engine": ins["engine"],
                                "ins": [],
                                "name": f"{ins['name']}-wsplit{i}",
                                "opcode": "NoOp",
                                "outs": [],
                                "sync_info": {
                                    "on_update": [],
                                    "on_wait": excess[i : i + max_waits],
                                },
                            }
                        )
                    si["on_wait"] = keep
                new.append(ins)
            bb["instructions"] = new
    if not changed:
        return bir_json
    return json.dumps(d).encode()


def _patched_compile_bir_kernel(bir_json, tmpdir, neff_name="file.neff"):
    return compile_bir_kernel(_split_excess_waits(bir_json), tmpdir, neff_name)


bass2jax.compile_bir_kernel = _patched_compile_bir_kernel

BF16 = ml_dtypes.bfloat16
F32 = mybir.dt.float32
BF = mybir.dt.bfloat16

DIM = 2048
T = 2048
HD = 64
N_CORES = 8
AF = mybir.ActivationFunctionType


class _TileContext(tile.TileContext):
    """TileContext whose final drain carries one sem wait per instruction.

    The walrus build in this image rejects a Drain carrying several sync
    waits ("Too many sync wait commands"), so emit individual single-wait
    NOPs on the sync engine first, then an unadorned drain + barriers.
    """

    def _drain_and_barrier(self, tick_clock, wait_clock):
        gc = tick_clock.global_clock
        vals = eval(repr(gc).replace("VectorClock(", "").rstrip(")"))
        for i, v in enumerate(vals):
            if v:
                single = [0] * len(vals)
                single[i] = v
                nop = self.nc.sync.nop(nofuse=True)
                wait_clock.add_sem_waits(
                    nop.ins, ScopedClock({None: VectorClock(single)})
                )
        self.nc.sync.drain()
        self.nc.all_engine_barrier()
        popped = self.nc._tile_sem_poison_stack.pop()
        assert popped is self._sem_poison
        self.nc.clear_and_free_semaphores(list(self.sems.allocated().values()))
        self.nc.all_engine_barrier()


def _build_nc():
    import os
    STAGE = int(os.environ.get("KSTAGE", "9"))
    nc = bass.Bass("TRN2")

    xt = nc.declare_dram_parameter("xt", [DIM, T], BF, isOutput=False)
    wq = nc.declare_dram_parameter("wq", [DIM, 512], BF, isOutput=False)
    wkv = nc.declare_dram_parameter("wkv", [DIM, 128], BF, isOutput=False)
    wo = nc.declare_dram_parameter("wo", [DIM, 512], BF, isOutput=False)
    cos2 = nc.declare_dram_parameter("cos2", [128, T], BF, isOutput=False)
    sin2 = nc.declare_dram_parameter("sin2", [128, T], BF, isOutput=False)
    coskv = nc.declare_dram_parameter("coskv", [128, T], BF, isOutput=False)
    sinkv = nc.declare_dram_parameter("sinkv", [128, T], BF, isOutput=False)
    r2t = nc.declare_dram_parameter("r2t", [128, 128], BF, isOutput=False)
    ident2 = nc.declare_dram_parameter("ident2", [128, 64], BF, isOutput=False)
    masks = nc.declare_dram_parameter("masks", [128, T], BF, isOutput=False)
    outt = nc.declare_dram_parameter("outt", [512, T], F32, isOutput=True)

    with _TileContext(nc) as tc:
        with (
            tc.tile_pool(name="consts", bufs=1) as consts,
            tc.tile_pool(name="big", bufs=1) as big,
            tc.tile_pool(name="wts", bufs=1) as wts,
            tc.tile_pool(name="acts", bufs=1) as acts,
            tc.tile_pool(name="work", bufs=4) as work,
            tc.tile_pool(name="exps", bufs=6) as exps,
            tc.tile_pool(name="outp", bufs=3) as outp,
            tc.tile_pool(name="psum", bufs=3, space="PSUM") as psum,
            tc.tile_pool(name="dram", bufs=1, space="DRAM") as dram,
        ):
            # ---- constants ----
            cos2_sb = consts.tile([128, T], BF)
            nc.sync.dma_start(cos2_sb[:], cos2[:])
            sin2_sb = consts.tile([128, T], BF)
            nc.sync.dma_start(sin2_sb[:], sin2[:])
            coskv_sb = consts.tile([128, T], BF)
            nc.sync.dma_start(coskv_sb[:], coskv[:])
            sinkv_sb = consts.tile([128, T], BF)
            nc.sync.dma_start(sinkv_sb[:], sinkv[:])
            masks_sb = consts.tile([128, T], BF)
            nc.sync.dma_start(masks_sb[:], masks[:])
            r2t_sb = consts.tile([128, 128], BF)
            nc.sync.dma_start(r2t_sb[:], r2t[:])
            ident2_sb = consts.tile([128, 64], BF)
            nc.sync.dma_start(ident2_sb[:], ident2[:])

            # ---- activations / weights in ----
            xt_sb = big.tile([128, 16, T], BF, tag="big")
            for fc in range(16):
                nc.sync.dma_start(xt_sb[:, fc, :], xt[fc * 128 : fc * 128 + 128, :])
            wkv_sb = wts.tile([128, 16, 128], BF)
            for fc in range(16):
                nc.sync.dma_start(
                    wkv_sb[:, fc, :], wkv[fc * 128 : fc * 128 + 128, :]
                )
            wq_sb = wts.tile([128, 16, 4, 128], BF)
            for fc in range(16):
                nc.sync.dma_start(
                    wq_sb[:, fc, :, :],
                    wq[fc * 128 : fc * 128 + 128, :].rearrange(
                        "p (qc m) -> p qc m", m=128
                    ),
                )
            wo_sb = wts.tile([128, 16, 4, 128], BF)
            for fc in range(16):
                nc.sync.dma_start(
                    wo_sb[:, fc, :, :],
                    wo[fc * 128 : fc * 128 + 128, :].rearrange(
                        "p (cc m) -> p cc m", m=128
                    ),
                )

            # ---- kv projection + rope (k rows 0..63, v rows 64..127) ----
            kvrope_sb = acts.tile([128, T], BF)
            for tt in range(4 if STAGE >= 2 else 0):
                ts = slice(tt * 512, tt * 512 + 512)
                ps = psum.tile([128, 512], F32, tag="mm")
                for fc in range(16):
                    nc.tensor.matmul(
                        ps[:],
                        lhsT=wkv_sb[:, fc, :],
                        rhs=xt_sb[:, fc, ts],
                        start=(fc == 0),
                        stop=(fc == 15),
                    )
                kv_sb = work.tile([128, 512], BF, tag="evac")
                nc.vector.tensor_copy(kv_sb[:], ps[:])
                psu = psum.tile([128, 512], F32, tag="mm", name="psu")
                nc.tensor.matmul(
                    psu[:], lhsT=r2t_sb[:], rhs=kv_sb[:], start=True, stop=True
                )
                t1 = work.tile([128, 512], BF, tag="t1")
                nc.vector.tensor_mul(t1[:], kv_sb[:], coskv_sb[:, ts])
                t2 = work.tile([128, 512], BF, tag="t2")
                nc.vector.tensor_mul(t2[:], psu[:], sinkv_sb[:, ts])
                nc.vector.tensor_add(kvrope_sb[:, ts], t1[:], t2[:])

            # duplicate roped k into both partition halves (row-group packing)
            kdup_sb = acts.tile([128, T], BF)
            if STAGE >= 2:
                nc.sync.dma_start(kdup_sb[0:64, :], kvrope_sb[0:64, :])
                nc.sync.dma_start(kdup_sb[64:128, :], kvrope_sb[0:64, :])

            # v' chunks [128 tok, 65]: col 64 = 1.0 (softmax denominator trick)
            v1_sb = acts.tile([128, 16, 65], BF)
            nc.vector.memset(v1_sb[:, :, 64:65], 1.0)
            for kt in range(16 if STAGE >= 2 else 0):
                pst = psum.tile([128, 64], BF, tag="pav", bufs=2)
                nc.tensor.transpose(
                    pst[:],
                    kvrope_sb[64:128, kt * 128 : kt * 128 + 128],
                    ident2_sb[64:128, :],
                )
                nc.scalar.copy(v1_sb[:, kt, 0:64], pst[:])

            # ---- q projection chunks interleaved with attention head pairs ----
            qrope_sb = acts.tile([128, 4, T], BF)
            ao_q = [
                dram.tile([128, T], BF, name=f"aoq{i}") for i in range(4)
            ]
            aof_q = [
                dram.tile([512, T], BF, name=f"aofq{i}") for i in range(4)
            ]
            scale = 1.0 / np.sqrt(HD)
            aof_sb = big.tile([128, 16, T], BF, tag="big")

            for ph in range(4):

                if ph == 3 and STAGE >= 6:
                    # reload already-gathered quarters while ph3 computes
                    # (gpsimd queue is idle; these wait only for xt release)
                    for i in range(3):
                        for c in range(4):
                            nc.gpsimd.dma_start(
                                aof_sb[:, 4 * i + c, :],
                                aof_q[i][c * 128 : c * 128 + 128, :],
                            )
                if STAGE >= 3:
                  for tt in range(4):
                    ts = slice(tt * 512, tt * 512 + 512)
                    ps = psum.tile([128, 512], F32, tag="mm", name="psq")
                    for fc in range(16):
                        nc.tensor.matmul(
                            ps[:],
                            lhsT=wq_sb[:, fc, ph, :],
                            rhs=xt_sb[:, fc, ts],
                            start=(fc == 0),
                            stop=(fc == 15),
                        )
                    q_sb = work.tile([128, 512], BF, tag="evac")
                    nc.vector.tensor_copy(q_sb[:], ps[:])
                    psu = psum.tile([128, 512], F32, tag="mm", name="psu2")
                    nc.tensor.matmul(
                        psu[:], lhsT=r2t_sb[:], rhs=q_sb[:],
                        start=True, stop=True,
                    )
                    t1 = work.tile([128, 512], BF, tag="t1")
                    nc.vector.tensor_mul(t1[:], q_sb[:], cos2_sb[:, ts])
                    t2 = work.tile([128, 512], BF, tag="t2")
                    nc.vector.tensor_mul(t2[:], psu[:], sin2_sb[:, ts])
                    nc.vector.tensor_add(qrope_sb[:, ph, ts], t1[:], t2[:])

                for qb in range(4):
                    # attention for heads (2*ph, 2*ph+1), query tile qb
                    if STAGE < 4:
                        continue
                    Q0 = qb * 512
                    nkt = 4 * qb + 4
                    pav = [
                        psum.tile([65, 512], F32, tag="pav", name=f"pav{i}", bufs=2)
                        for i in range(2)
                    ]
                    for pr in range(nkt // 2):
                        kt0, kt1 = 2 * pr, 2 * pr + 1
                        # causal-active widths (tiles above the diagonal shrink)
                        j0, j1 = kt0 - 4 * qb, kt1 - 4 * qb
                        w0 = 512 if j0 < 0 else 512 - 128 * j0
                        w1 = 512 if j1 < 0 else 512 - 128 * j1
                        diag = j0 >= 0
                        # scores for both head halves interleaved so adjacent
                        # matmuls target different PE row groups (concurrent)
                        pss = [
                            psum.tile([128, 1024], F32, tag="mm", name=f"pss{i}")
                            for i in range(2)
                        ]
                        for kt, w, off in ((kt0, w0, 0), (kt1, w1, w0)):
                            for par in range(2):
                                lo, hi = (0, 64) if par == 0 else (64, 128)
                                nc.tensor.matmul(
                                    pss[par][:, off : off + w],
                                    lhsT=kdup_sb[lo:hi, kt * 128 : kt * 128 + 128],
                                    rhs=qrope_sb[lo:hi, ph, Q0 + 512 - w : Q0 + 512],
                                    start=True,
                                    stop=True,
                                )
                        e_pair = []
                        for par in range(2):
                            e_sb = exps.tile([128, 1024], BF, tag="e", name=f"e{par}")
                            nc.scalar.activation(
                                e_sb[:, 0 : w0 + w1], pss[par][:, 0 : w0 + w1],
                                AF.Exp, scale=scale,
                            )
                            if diag:
                                nc.vector.tensor_mul(
                                    e_sb[:, 0:w0], e_sb[:, 0:w0],
                                    masks_sb[:, 0:w0],
                                )
                                nc.vector.tensor_mul(
                                    e_sb[:, w0 : w0 + w1], e_sb[:, w0 : w0 + w1],
                                    masks_sb[:, 0:w1],
                                )
                            e_pair.append(e_sb)
                        for kt, w, off in ((kt0, w0, 0), (kt1, w1, w0)):
                            for par in range(2):
                                nc.tensor.matmul(
                                    pav[par][:, 512 - w : 512],
                                    lhsT=v1_sb[:, kt, :],
                                    rhs=e_pair[par][:, off : off + w],
                                    start=(kt == 0),
                                    stop=(kt == nkt - 1),
                                )
                    # evacuate unnormalized av + denominators (one copy per
                    # half), releasing the PSUM accumulators immediately; the
                    # normalization below runs off the critical path with no
                    # PE/PSUM involvement (DRAM-bounce broadcast), and both
                    # halves share one 128-lane reciprocal
                    avu = []
                    for par in range(2):
                        avu_sb = work.tile([65, 512], BF, tag="avu", name=f"avu{par}")
                        nc.scalar.copy(avu_sb[:], pav[par][:])
                        avu.append(avu_sb)
                    dden = dram.tile([2, 512], BF, tag="dden", bufs=4, name="dden")
                    for par in range(2):
                        nc.sync.dma_start(dden[par : par + 1, :], avu[par][64:65, :])
                    rden_sb = work.tile([128, 8], BF, tag="rden")
                    nc.sync.dma_start(
                        rden_sb[:],
                        bass.AP(tensor=dden.tensor, offset=dden.offset,
                                ap=[[8, 128], [1, 8]]),
                    )
                    with nc.allow_low_precision(
                        reason="bf16 softmax denominators are within tolerance"
                    ):
                        nc.vector.reciprocal(rden_sb[:], rden_sb[:])
                    rdden = dram.tile([2, 512], BF, tag="rdden", bufs=4, name="rdden")
                    nc.sync.dma_start(
                        bass.AP(tensor=rdden.tensor, offset=rdden.offset,
                                ap=[[8, 128], [1, 8]]),
                        rden_sb[:],
                    )
                    for par in range(2):
                        b_sb = work.tile([64, 512], BF, tag="bcast", name=f"b{par}")
                        nc.sync.dma_start(
                            b_sb[:],
                            bass.AP(
                                tensor=rdden.tensor,
                                offset=rdden[par : par + 1, :].offset,
                                ap=[[0, 64], [1, 512]],
                            ),
                        )
                        av_sb = work.tile([64, 512], BF, tag="av", name=f"av{par}")
                        nc.vector.tensor_mul(
                            av_sb[:], avu[par][0:64, :], b_sb[:]
                        )
                        nc.sync.dma_start(
                            ao_q[ph][64 * par : 64 * par + 64, Q0 : Q0 + 512],
                            av_sb[:],
                        )
                if STAGE >= 5:
                    # gather this head pair while later pairs compute
                    nc.gpsimd.collective_compute(
                        "AllGather",
                        mybir.AluOpType.bypass,
                        ins=[ao_q[ph].opt()],
                        outs=[aof_q[ph].opt()],
                        replica_groups=[[0, 1, 2, 3], [4, 5, 6, 7]],
                    )
                    if STAGE >= 6 and ph == 3:
                        for c in range(4):
                            nc.sync.dma_start(
                                aof_sb[:, 12 + c, :],
                                aof_q[3][c * 128 : c * 128 + 128, :],
                            )

            # ---- output projection (512-column slice of wo) ----
            # aof_q[i] chunk c covers rank c, head pair i of that rank
            #   -> wo feature-chunk 4*c + i
            NWO = int(os.environ.get('KWO', '4'))
            for cc in range(NWO if STAGE >= 7 else 0):
                for tt in range(4):
                    ts = slice(tt * 512, tt * 512 + 512)
                    ps = psum.tile([128, 512], F32, tag="mm")
                    for i in range(4):
                        for c in range(4):
                            nc.tensor.matmul(
                                ps[:],
                                lhsT=wo_sb[:, 4 * c + i, cc, :],
                                rhs=aof_sb[:, 4 * i + c, ts],
                                start=(i == 0 and c == 0),
                                stop=(i == 3 and c == 3),
                            )
                    o_sb = outp.tile([128, 512], F32, tag="o")
                    nc.scalar.copy(o_sb[:], ps[:])
                    nc.sync.dma_start(outt[cc * 128 : cc * 128 + 128, ts], o_sb[:])

    return nc


def _host_tables():
    inv_freq = 1.0 / (10000.0 ** (np.arange(0, HD, 2, dtype=np.float32) / HD))
    t = np.arange(T, dtype=np.float32)
    freqs = np.einsum("i,j->ij", t, inv_freq)
    emb = np.concatenate([freqs, freqs], axis=-1)  # [T, 64]
    cosT = np.cos(emb).T.astype(np.float32)  # [64, T]
    sinT = np.sin(emb).T.astype(np.float32)

    cos2 = np.ascontiguousarray(np.vstack([cosT, cosT])).astype(BF16)
    sin2 = np.ascontiguousarray(np.vstack([sinT, sinT])).astype(BF16)
    coskv = np.ascontiguousarray(np.vstack([cosT, np.ones_like(cosT)])).astype(BF16)
    sinkv = np.ascontiguousarray(np.vstack([sinT, np.zeros_like(sinT)])).astype(BF16)

    R = np.zeros((HD, HD), dtype=np.float32)
    for d in range(32):
        R[d, d + 32] = -1.0
        R[d + 32, d] = 1.0
    r2 = np.block([[R, np.zeros_like(R)], [np.zeros_like(R), R]])
    r2t = np.ascontiguousarray(r2.T).astype(BF16)  # lhsT: matmul computes R2 @ rhs

    ident2 = np.vstack([np.eye(HD), np.eye(HD)]).astype(BF16)  # [128, 64]

    masks = np.zeros((128, T), dtype=np.float32)
    r_idx = np.arange(128)[:, None]
    c_idx = np.arange(512)[None, :]
    for j in range(4):
        masks[:, j * 512 : j * 512 + 512] = (c_idx >= 128 * j + r_idx)
    masks = masks.astype(BF16)

    return dict(
        cos2=cos2, sin2=sin2, coskv=coskv, sinkv=sinkv,
        r2t=r2t, ident2=ident2, masks=masks,
    )


_STATE = {}


def _get_nc():
    if "nc" not in _STATE:
        _STATE["nc"] = _build_nc()
        _STATE["tables"] = _host_tables()
    return _STATE["nc"], _STATE["tables"]


def kernel(x, wq, wk, wv, wo):
    nc, tables = _get_nc()

    x = np.asarray(x, dtype=np.float32)
    wq_b = np.asarray(wq, dtype=np.float32).astype(BF16)
    wo_b = np.asarray(wo, dtype=np.float32).astype(BF16)
    wk_b = np.asarray(wk, dtype=np.float32).astype(BF16)
    wv_b = np.asarray(wv, dtype=np.float32).astype(BF16)

    in_maps = []
    xt_b = [np.ascontiguousarray(x[b].T).astype(BF16) for b in range(2)]
    for core in range(N_CORES):
        b, g = core // 4, core % 4
        m = dict(tables)
        m["xt"] = xt_b[b]
        m["wq"] = np.ascontiguousarray(wq_b[:, 512 * g : 512 * g + 512])
        m["wkv"] = np.ascontiguousarray(
            np.concatenate(
                [wk_b[:, 64 * g : 64 * g + 64], wv_b[:, 64 * g : 64 * g + 64]],
                axis=1,
            )
        )
        m["wo"] = np.ascontiguousarray(wo_b[:, 512 * g : 512 * g + 512])
        in_maps.append(m)

    res = run_bass_kernel_spmd(
        nc, in_maps, core_ids=list(range(N_CORES)), trace=False
    )

    out = np.empty((2, T, DIM), dtype=np.float32)
    for core in range(N_CORES):
        b, g = core // 4, core % 4
        out[b][:, 512 * g : 512 * g + 512] = res.results[core]["outt"].T
    return out


# revision 35
# speedup vs baseline: 1.1015x; 1.0048x over previous
"""Distributed GQA attention block (dense_transformer) for 8 TRN2 NeuronCores.

Reference computation (all fp32):
    q = (x @ wq)  -> RoPE;  k = (x @ wk) -> RoPE;  v = x @ wv
    causal softmax(q k^T / sqrt(64)) @ v  (GQA: 32 q heads, 4 kv heads)
    out = attn_out @ wo

Sharding: core (b, g) for b in {0,1}, g in {0..3} handles batch b, q-heads
8g..8g+7, kv-head g (data-parallel over batch x tensor-parallel over GQA
groups).  Each core computes attn_outT for its heads ([512, 2048],
feature-major), AllGathers within its 4-core batch group, and applies a
512-column slice of wo.  Outputs are disjoint -> host concat only.

All activations/weights are kept feature-major (transposed) on chip so every
matmul contracts over the partition dim with no on-chip transposes except a
single small one for v.  Matmul compute in bf16 (fp32 PSUM accumulate).
"""

import json

import numpy as np
import ml_dtypes

import concourse.bass as bass
import concourse.bass2jax as bass2jax
import concourse.mybir as mybir
import concourse.tile as tile
from concourse.tile import VectorClock, ScopedClock
from concourse.bass_utils import compile_bir_kernel, run_bass_kernel_spmd

_MAX_WAITS = 1  # this walrus build rejects instructions with more sem waits


def _split_excess_waits(bir_json, max_waits=_MAX_WAITS):
    """Hoist excess per-instruction sem waits onto injected same-engine NoOps.

    The TRN2 ISA encoding in this neuronxcc build allows at most `max_waits`
    sync-wait commands per instruction; Tile's sem assigner can emit more.
    A NoOp inserted immediately before the instruction on the same engine is
    semantically identical (the engine blocks at the same program point).
    """
    d = json.loads(bir_json)
    changed = False
    for fn in d.get("functions", []):
        for bb in fn.get("blocks", []):
            insts = bb.get("instructions", [])
            new = []
            for ins in insts:
                si = ins.get("sync_info")
                waits = (si or {}).get("on_wait") or []
                if len(waits) > max_waits:
                    changed = True
                    excess, keep = waits[:-max_waits], waits[-max_waits:]
                    for i in range(0, len(excess), max_waits):
                        new.append(
                            {
                                "debug": ins.get("debug", 0),
                                "engine": ins["engine"],
                                "ins": [],
                                "name": f"{ins['name']}-wsplit{i}",
                                "opcode": "NoOp",
                                "outs": [],
                                "sync_info": {
                                    "on_update": [],
                                    "on_wait": excess[i : i + max_waits],
                                },
                            }
                        )
                    si["on_wait"] = keep
                new.append(ins)
            bb["instructions"] = new
    if not changed:
        return bir_json
    return json.dumps(d).encode()


def _patched_compile_bir_kernel(bir_json, tmpdir, neff_name="file.neff"):
    return compile_bir_kernel(_split_excess_waits(bir_json), tmpdir, neff_name)


bass2jax.compile_bir_kernel = _patched_compile_bir_kernel

BF16 = ml_dtypes.bfloat16
F32 = mybir.dt.float32
BF = mybir.dt.bfloat16

DIM = 2048
T = 2048
HD = 64
N_CORES = 8
AF = mybir.ActivationFunctionType


class _TileContext(tile.TileContext):
    """TileContext whose final drain carries one sem wait per instruction.

    The walrus build in this image rejects a Drain carrying several sync
    waits ("Too many sync wait commands"), so emit individual single-wait
    NOPs on the sync engine first, then an unadorned drain + barriers.
    """

    def _drain_and_barrier(self, tick_clock, wait_clock):
        gc = tick_clock.global_clock
        vals = eval(repr(gc).replace("VectorClock(", "").rstrip(")"))
        for i, v in enumerate(vals):
            if v:
                single = [0] * len(vals)
                single[i] = v
                nop = self.nc.sync.nop(nofuse=True)
                wait_clock.add_sem_waits(
                    nop.ins, ScopedClock({None: VectorClock(single)})
                )
        self.nc.sync.drain()
        self.nc.all_engine_barrier()
        popped = self.nc._tile_sem_poison_stack.pop()
        assert popped is self._sem_poison
        self.nc.clear_and_free_semaphores(list(self.sems.allocated().values()))
        self.nc.all_engine_barrier()


def _build_nc():
    import os
    STAGE = int(os.environ.get("KSTAGE", "9"))
    nc = bass.Bass("TRN2")

    xt = nc.declare_dram_parameter("xt", [DIM, T], BF, isOutput=False)
    wq = nc.declare_dram_parameter("wq", [DIM, 512], BF, isOutput=False)
    wkv = nc.declare_dram_parameter("wkv", [DIM, 128], BF, isOutput=False)
    wo = nc.declare_dram_parameter("wo", [DIM, 512], BF, isOutput=False)
    cos2 = nc.declare_dram_parameter("cos2", [128, T], BF, isOutput=False)
    sin2 = nc.declare_dram_parameter("sin2", [128, T], BF, isOutput=False)
    coskv = nc.declare_dram_parameter("coskv", [128, T], BF, isOutput=False)
    sinkv = nc.declare_dram_parameter("sinkv", [128, T], BF, isOutput=False)
    r2t = nc.declare_dram_parameter("r2t", [128, 128], BF, isOutput=False)
    ident2 = nc.declare_dram_parameter("ident2", [128, 64], BF, isOutput=False)
    masks = nc.declare_dram_parameter("masks", [128, T], BF, isOutput=False)
    outt = nc.declare_dram_parameter("outt", [512, T], F32, isOutput=True)

    with _TileContext(nc) as tc:
        with (
            tc.tile_pool(name="consts", bufs=1) as consts,
            tc.tile_pool(name="big", bufs=1) as big,
            tc.tile_pool(name="wts", bufs=1) as wts,
            tc.tile_pool(name="acts", bufs=1) as acts,
            tc.tile_pool(name="work", bufs=4) as work,
            tc.tile_pool(name="exps", bufs=6) as exps,
            tc.tile_pool(name="outp", bufs=3) as outp,
            tc.tile_pool(name="psum", bufs=3, space="PSUM") as psum,
            tc.tile_pool(name="dram", bufs=1, space="DRAM") as dram,
        ):
            # ---- constants ----
            cos2_sb = consts.tile([128, T], BF)
            nc.sync.dma_start(cos2_sb[:], cos2[:])
            sin2_sb = consts.tile([128, T], BF)
            nc.sync.dma_start(sin2_sb[:], sin2[:])
            coskv_sb = consts.tile([128, T], BF)
            nc.sync.dma_start(coskv_sb[:], coskv[:])
            sinkv_sb = consts.tile([128, T], BF)
            nc.sync.dma_start(sinkv_sb[:], sinkv[:])
            masks_sb = consts.tile([128, T], BF)
            nc.sync.dma_start(masks_sb[:], masks[:])
            r2t_sb = consts.tile([128, 128], BF)
            nc.sync.dma_start(r2t_sb[:], r2t[:])
            ident2_sb = consts.tile([128, 64], BF)
            nc.sync.dma_start(ident2_sb[:], ident2[:])

            # ---- activations / weights in ----
            xt_sb = big.tile([128, 16, T], BF, tag="big")
            for fc in range(16):
                nc.sync.dma_start(xt_sb[:, fc, :], xt[fc * 128 : fc * 128 + 128, :])
            wkv_sb = wts.tile([128, 16, 128], BF)
            for fc in range(16):
                nc.sync.dma_start(
                    wkv_sb[:, fc, :], wkv[fc * 128 : fc * 128 + 128, :]
                )
            wq_sb = wts.tile([128, 16, 4, 128], BF)
            for fc in range(16):
                nc.sync.dma_start(
                    wq_sb[:, fc, :, :],
                    wq[fc * 128 : fc * 128 + 128, :].rearrange(
                        "p (qc m) -> p qc m", m=128
                    ),
                )
            wo_sb = wts.tile([128, 16, 4, 128], BF)

            # ---- kv projection + rope (k rows 0..63, v rows 64..127) ----
            kvrope_sb = acts.tile([128, T], BF)
            for tt in range(4 if STAGE >= 2 else 0):
                ts = slice(tt * 512, tt * 512 + 512)
                ps = psum.tile([128, 512], F32, tag="mm")
                for fc in range(16):
                    nc.tensor.matmul(
                        ps[:],
                        lhsT=wkv_sb[:, fc, :],
                        rhs=xt_sb[:, fc, ts],
                        start=(fc == 0),
                        stop=(fc == 15),
                    )
                kv_sb = work.tile([128, 512], BF, tag="evac")
                nc.vector.tensor_copy(kv_sb[:], ps[:])
                psu = psum.tile([128, 512], F32, tag="mm", name="psu")
                nc.tensor.matmul(
                    psu[:], lhsT=r2t_sb[:], rhs=kv_sb[:], start=True, stop=True
                )
                t1 = work.tile([128, 512], BF, tag="t1")
                nc.vector.tensor_mul(t1[:], kv_sb[:], coskv_sb[:, ts])
                t2 = work.tile([128, 512], BF, tag="t2")
                nc.vector.tensor_mul(t2[:], psu[:], sinkv_sb[:, ts])
                nc.vector.tensor_add(kvrope_sb[:, ts], t1[:], t2[:])

            # duplicate roped k into both partition halves (row-group packing)
            kdup_sb = acts.tile([128, T], BF)
            if STAGE >= 2:
                nc.sync.dma_start(kdup_sb[0:64, :], kvrope_sb[0:64, :])
                nc.sync.dma_start(kdup_sb[64:128, :], kvrope_sb[0:64, :])

            # v' chunks [128 tok, 65]: col 64 = 1.0 (softmax denominator trick)
            v1_sb = acts.tile([128, 16, 65], BF)
            nc.vector.memset(v1_sb[:, :, 64:65], 1.0)
            for kt in range(16 if STAGE >= 2 else 0):
                pst = psum.tile([128, 64], BF, tag="pav", bufs=2)
                nc.tensor.transpose(
                    pst[:],
                    kvrope_sb[64:128, kt * 128 : kt * 128 + 128],
                    ident2_sb[64:128, :],
                )
                nc.scalar.copy(v1_sb[:, kt, 0:64], pst[:])

            # ---- q projection chunks interleaved with attention head pairs ----
            qrope_sb = acts.tile([128, 4, T], BF)
            ao_q = [
                dram.tile([128, T], BF, name=f"aoq{i}") for i in range(4)
            ]
            aof_q = [
                dram.tile([512, T], BF, name=f"aofq{i}") for i in range(4)
            ]
            scale = 1.0 / np.sqrt(HD)
            aof_sb = big.tile([128, 16, T], BF, tag="big")

            for ph in range(4):

                if ph == 3 and STAGE >= 6:
                    # reload already-gathered quarters while ph3 computes
                    # (gpsimd queue is idle; these wait only for xt release)
                    for i in range(3):
                        for c in range(4):
                            nc.gpsimd.dma_start(
                                aof_sb[:, 4 * i + c, :],
                                aof_q[i][c * 128 : c * 128 + 128, :],
                            )
                if STAGE >= 3:
                  for tt in range(4):
                    ts = slice(tt * 512, tt * 512 + 512)
                    ps = psum.tile([128, 512], F32, tag="mm", name="psq")
                    for fc in range(16):
                        nc.tensor.matmul(
                            ps[:],
                            lhsT=wq_sb[:, fc, ph, :],
                            rhs=xt_sb[:, fc, ts],
                            start=(fc == 0),
                            stop=(fc == 15),
                        )
                    q_sb = work.tile([128, 512], BF, tag="evac")
                    nc.vector.tensor_copy(q_sb[:], ps[:])
                    psu = psum.tile([128, 512], F32, tag="mm", name="psu2")
                    nc.tensor.matmul(
                        psu[:], lhsT=r2t_sb[:], rhs=q_sb[:],
                        start=True, stop=True,
                    )
                    t1 = work.tile([128, 512], BF, tag="t1")
                    nc.vector.tensor_mul(t1[:], q_sb[:], cos2_sb[:, ts])
                    t2 = work.tile([128, 512], BF, tag="t2")
                    nc.vector.tensor_mul(t2[:], psu[:], sin2_sb[:, ts])
                    nc.vector.tensor_add(qrope_sb[:, ph, ts], t1[:], t2[:])

                for qb in range(4):
                    # attention for heads (2*ph, 2*ph+1), query tile qb
                    if STAGE < 4:
                        continue
                    Q0 = qb * 512
                    nkt = 4 * qb + 4
                    pav = [
                        psum.tile([65, 512], F32, tag="pav", name=f"pav{i}", bufs=2)
                        for i in range(2)
                    ]
                    for pr in range(nkt // 2):
                        kt0, kt1 = 2 * pr, 2 * pr + 1
                        # causal-active widths (tiles above the diagonal shrink)
                        j0, j1 = kt0 - 4 * qb, kt1 - 4 * qb
                        w0 = 512 if j0 < 0 else 512 - 128 * j0
                        w1 = 512 if j1 < 0 else 512 - 128 * j1
                        diag = j0 >= 0
                        # scores for both head halves interleaved so adjacent
                        # matmuls target different PE row groups (concurrent)
                        pss = [
                            psum.tile([128, 1024], F32, tag="mm", name=f"pss{i}")
                            for i in range(2)
                        ]
                        for kt, w, off in ((kt0, w0, 0), (kt1, w1, w0)):
                            for par in range(2):
                                lo, hi = (0, 64) if par == 0 else (64, 128)
                                nc.tensor.matmul(
                                    pss[par][:, off : off + w],
                                    lhsT=kdup_sb[lo:hi, kt * 128 : kt * 128 + 128],
                                    rhs=qrope_sb[lo:hi, ph, Q0 + 512 - w : Q0 + 512],
                                    start=True,
                                    stop=True,
                                )
                        e_pair = []
                        for par in range(2):
                            e_sb = exps.tile([128, 1024], BF, tag="e", name=f"e{par}")
                            nc.scalar.activation(
                                e_sb[:, 0 : w0 + w1], pss[par][:, 0 : w0 + w1],
                                AF.Exp, scale=scale,
                            )
                            if diag:
                                nc.vector.tensor_mul(
                                    e_sb[:, 0:w0], e_sb[:, 0:w0],
                                    masks_sb[:, 0:w0],
                                )
                                nc.vector.tensor_mul(
                                    e_sb[:, w0 : w0 + w1], e_sb[:, w0 : w0 + w1],
                                    masks_sb[:, 0:w1],
                                )
                            e_pair.append(e_sb)
                        for kt, w, off in ((kt0, w0, 0), (kt1, w1, w0)):
                            for par in range(2):
                                nc.tensor.matmul(
                                    pav[par][:, 512 - w : 512],
                                    lhsT=v1_sb[:, kt, :],
                                    rhs=e_pair[par][:, off : off + w],
                                    start=(kt == 0),
                                    stop=(kt == nkt - 1),
                                )
                    # evacuate unnormalized av + denominators (one copy per
                    # half), releasing the PSUM accumulators immediately; the
                    # normalization below runs off the critical path with no
                    # PE/PSUM involvement (DRAM-bounce broadcast), and both
                    # halves share one 128-lane reciprocal
                    avu = []
                    for par in range(2):
                        avu_sb = work.tile([65, 512], BF, tag="avu", name=f"avu{par}")
                        nc.scalar.copy(avu_sb[:], pav[par][:])
                        avu.append(avu_sb)
                    dden = dram.tile([2, 512], BF, tag="dden", bufs=4, name="dden")
                    for par in range(2):
                        nc.sync.dma_start(dden[par : par + 1, :], avu[par][64:65, :])
                    rden_sb = work.tile([128, 8], BF, tag="rden")
                    nc.sync.dma_start(
                        rden_sb[:],
                        bass.AP(tensor=dden.tensor, offset=dden.offset,
                                ap=[[8, 128], [1, 8]]),
                    )
                    with nc.allow_low_precision(
                        reason="bf16 softmax denominators are within tolerance"
                    ):
                        nc.vector.reciprocal(rden_sb[:], rden_sb[:])
                    rdden = dram.tile([2, 512], BF, tag="rdden", bufs=4, name="rdden")
                    nc.sync.dma_start(
                        bass.AP(tensor=rdden.tensor, offset=rdden.offset,
                                ap=[[8, 128], [1, 8]]),
                        rden_sb[:],
                    )
                    for par in range(2):
                        b_sb = work.tile([64, 512], BF, tag="bcast", name=f"b{par}")
                        nc.sync.dma_start(
                            b_sb[:],
                            bass.AP(
                                tensor=rdden.tensor,
                                offset=rdden[par : par + 1, :].offset,
                                ap=[[0, 64], [1, 512]],
                            ),
                        )
                        av_sb = work.tile([64, 512], BF, tag="av", name=f"av{par}")
                        nc.vector.tensor_mul(
                            av_sb[:], avu[par][0:64, :], b_sb[:]
                        )
                        nc.sync.dma_start(
                            ao_q[ph][64 * par : 64 * par + 64, Q0 : Q0 + 512],
                            av_sb[:],
                        )
                if STAGE >= 5:
                    # gather this head pair while later pairs compute
                    nc.gpsimd.collective_compute(
                        "AllGather",
                        mybir.AluOpType.bypass,
                        ins=[ao_q[ph].opt()],
                        outs=[aof_q[ph].opt()],
                        replica_groups=[[0, 1, 2, 3], [4, 5, 6, 7]],
                    )
                    if STAGE >= 6 and ph == 3:
                        for c in range(4):
                            nc.sync.dma_start(
                                aof_sb[:, 12 + c, :],
                                aof_q[3][c * 128 : c * 128 + 128, :],
                            )

            # wo weights land right after attention, spread across three
            # DMA-issuing queues so the 4 MB arrives in a few microseconds
            _eng = [nc.sync, nc.scalar]
            for fc in range(16 if STAGE >= 7 else 0):
                _eng[fc % 2].dma_start(
                    wo_sb[:, fc, :, :],
                    wo[fc * 128 : fc * 128 + 128, :].rearrange(
                        "p (cc m) -> p cc m", m=128
                    ),
                )

            # ---- output projection (512-column slice of wo) ----
            # aof_q[i] chunk c covers rank c, head pair i of that rank
            #   -> wo feature-chunk 4*c + i
            NWO = int(os.environ.get('KWO', '4'))
            for cc in range(NWO if STAGE >= 7 else 0):
                for tt in range(4):
                    ts = slice(tt * 512, tt * 512 + 512)
                    ps = psum.tile([128, 512], F32, tag="mm")
                    for i in range(4):
                        for c in range(4):
                            nc.tensor.matmul(
                                ps[:],
                                lhsT=wo_sb[:, 4 * c + i, cc, :],
                                rhs=aof_sb[:, 4 * i + c, ts],
                                start=(i == 0 and c == 0),
                                stop=(i == 3 and c == 3),
                            )
                    o_sb = outp.tile([128, 512], F32, tag="o")
                    nc.scalar.copy(o_sb[:], ps[:])
                    nc.sync.dma_start(outt[cc * 128 : cc * 128 + 128, ts], o_sb[:])

    return nc


def _host_tables():
    inv_freq = 1.0 / (10000.0 ** (np.arange(0, HD, 2, dtype=np.float32) / HD))
    t = np.arange(T, dtype=np.float32)
    freqs = np.einsum("i,j->ij", t, inv_freq)
    emb = np.concatenate([freqs, freqs], axis=-1)  # [T, 64]
    cosT = np.cos(emb).T.astype(np.float32)  # [64, T]
    sinT = np.sin(emb).T.astype(np.float32)

    cos2 = np.ascontiguousarray(np.vstack([cosT, cosT])).astype(BF16)
    sin2 = np.ascontiguousarray(np.vstack([sinT, sinT])).astype(BF16)
    coskv = np.ascontiguousarray(np.vstack([cosT, np.ones_like(cosT)])).astype(BF16)
    sinkv = np.ascontiguousarray(np.vstack([sinT, np.zeros_like(sinT)])).astype(BF16)

    R = np.zeros((HD, HD), dtype=np.float32)
    for d in range(32):
        R[d, d + 32] = -1.0
        R[d + 32, d] = 1.0
    r2 = np.block([[R, np.zeros_like(R)], [np.zeros_like(R), R]])
    r2t = np.ascontiguousarray(r2.T).astype(BF16)  # lhsT: matmul computes R2 @ rhs

    ident2 = np.vstack([np.eye(HD), np.eye(HD)]).astype(BF16)  # [128, 64]

    masks = np.zeros((128, T), dtype=np.float32)
    r_idx = np.arange(128)[:, None]
    c_idx = np.arange(512)[None, :]
    for j in range(4):
        masks[:, j * 512 : j * 512 + 512] = (c_idx >= 128 * j + r_idx)
    masks = masks.astype(BF16)

    return dict(
        cos2=cos2, sin2=sin2, coskv=coskv, sinkv=sinkv,
        r2t=r2t, ident2=ident2, masks=masks,
    )


_STATE = {}


def _get_nc():
    if "nc" not in _STATE:
        _STATE["nc"] = _build_nc()
        _STATE["tables"] = _host_tables()
    return _STATE["nc"], _STATE["tables"]


def kernel(x, wq, wk, wv, wo):
    nc, tables = _get_nc()

    x = np.asarray(x, dtype=np.float32)
    wq_b = np.asarray(wq, dtype=np.float32).astype(BF16)
    wo_b = np.asarray(wo, dtype=np.float32).astype(BF16)
    wk_b = np.asarray(wk, dtype=np.float32).astype(BF16)
    wv_b = np.asarray(wv, dtype=np.float32).astype(BF16)

    in_maps = []
    xt_b = [np.ascontiguousarray(x[b].T).astype(BF16) for b in range(2)]
    for core in range(N_CORES):
        b, g = core // 4, core % 4
        m = dict(tables)
        m["xt"] = xt_b[b]
        m["wq"] = np.ascontiguousarray(wq_b[:, 512 * g : 512 * g + 512])
        m["wkv"] = np.ascontiguousarray(
            np.concatenate(
                [wk_b[:, 64 * g : 64 * g + 64], wv_b[:, 64 * g : 64 * g + 64]],
                axis=1,
            )
        )
        m["wo"] = np.ascontiguousarray(wo_b[:, 512 * g : 512 * g + 512])
        in_maps.append(m)

    res = run_bass_kernel_spmd(
        nc, in_maps, core_ids=list(range(N_CORES)), trace=False
    )

    out = np.empty((2, T, DIM), dtype=np.float32)
    for core in range(N_CORES):
        b, g = core // 4, core % 4
        out[b][:, 512 * g : 512 * g + 512] = res.results[core]["outt"].T
    return out


# revision 36
# speedup vs baseline: 1.1388x; 1.0339x over previous
"""Distributed GQA attention block (dense_transformer) for 8 TRN2 NeuronCores.

Reference computation (all fp32):
    q = (x @ wq)  -> RoPE;  k = (x @ wk) -> RoPE;  v = x @ wv
    causal softmax(q k^T / sqrt(64)) @ v  (GQA: 32 q heads, 4 kv heads)
    out = attn_out @ wo

Sharding: core (b, g) for b in {0,1}, g in {0..3} handles batch b, q-heads
8g..8g+7, kv-head g (data-parallel over batch x tensor-parallel over GQA
groups).  Each core computes attn_outT for its heads ([512, 2048],
feature-major), AllGathers within its 4-core batch group, and applies a
512-column slice of wo.  Outputs are disjoint -> host concat only.

All activations/weights are kept feature-major (transposed) on chip so every
matmul contracts over the partition dim with no on-chip transposes except a
single small one for v.  Matmul compute in bf16 (fp32 PSUM accumulate).
"""

import json

import numpy as np
import ml_dtypes

import concourse.bass as bass
import concourse.bass2jax as bass2jax
import concourse.mybir as mybir
import concourse.tile as tile
from concourse.tile import VectorClock, ScopedClock
from concourse.bass_utils import compile_bir_kernel, run_bass_kernel_spmd

_MAX_WAITS = 1  # this walrus build rejects instructions with more sem waits


def _split_excess_waits(bir_json, max_waits=_MAX_WAITS):
    """Hoist excess per-instruction sem waits onto injected same-engine NoOps.

    The TRN2 ISA encoding in this neuronxcc build allows at most `max_waits`
    sync-wait commands per instruction; Tile's sem assigner can emit more.
    A NoOp inserted immediately before the instruction on the same engine is
    semantically identical (the engine blocks at the same program point).
    """
    d = json.loads(bir_json)
    changed = False
    for fn in d.get("functions", []):
        for bb in fn.get("blocks", []):
            insts = bb.get("instructions", [])
            new = []
            for ins in insts:
                si = ins.get("sync_info")
                waits = (si or {}).get("on_wait") or []
                if len(waits) > max_waits:
                    changed = True
                    excess, keep = waits[:-max_waits], waits[-max_waits:]
                    for i in range(0, len(excess), max_waits):
                        new.append(
                            {
                                "debug": ins.get("debug", 0),
                                "engine": ins["engine"],
                                "ins": [],
                                "name": f"{ins['name']}-wsplit{i}",
                                "opcode": "NoOp",
                                "outs": [],
                                "sync_info": {
                                    "on_update": [],
                                    "on_wait": excess[i : i + max_waits],
                                },
                            }
                        )
                    si["on_wait"] = keep
                new.append(ins)
            bb["instructions"] = new
    if not changed:
        return bir_json
    return json.dumps(d).encode()


def _patched_compile_bir_kernel(bir_json, tmpdir, neff_name="file.neff"):
    return compile_bir_kernel(_split_excess_waits(bir_json), tmpdir, neff_name)


bass2jax.compile_bir_kernel = _patched_compile_bir_kernel

BF16 = ml_dtypes.bfloat16
F32 = mybir.dt.float32
BF = mybir.dt.bfloat16

DIM = 2048
T = 2048
HD = 64
N_CORES = 8
AF = mybir.ActivationFunctionType


class _TileContext(tile.TileContext):
    """TileContext whose final drain carries one sem wait per instruction.

    The walrus build in this image rejects a Drain carrying several sync
    waits ("Too many sync wait commands"), so emit individual single-wait
    NOPs on the sync engine first, then an unadorned drain + barriers.
    """

    def _drain_and_barrier(self, tick_clock, wait_clock):
        gc = tick_clock.global_clock
        vals = eval(repr(gc).replace("VectorClock(", "").rstrip(")"))
        for i, v in enumerate(vals):
            if v:
                single = [0] * len(vals)
                single[i] = v
                nop = self.nc.sync.nop(nofuse=True)
                wait_clock.add_sem_waits(
                    nop.ins, ScopedClock({None: VectorClock(single)})
                )
        self.nc.sync.drain()
        self.nc.all_engine_barrier()
        popped = self.nc._tile_sem_poison_stack.pop()
        assert popped is self._sem_poison
        self.nc.clear_and_free_semaphores(list(self.sems.allocated().values()))
        self.nc.all_engine_barrier()


def _build_nc():
    import os
    STAGE = int(os.environ.get("KSTAGE", "9"))
    nc = bass.Bass("TRN2")

    xt = nc.declare_dram_parameter("xt", [DIM, T], BF, isOutput=False)
    wq = nc.declare_dram_parameter("wq", [DIM, 512], BF, isOutput=False)
    wkv = nc.declare_dram_parameter("wkv", [DIM, 128], BF, isOutput=False)
    wo = nc.declare_dram_parameter("wo", [DIM, 512], BF, isOutput=False)
    cos2 = nc.declare_dram_parameter("cos2", [128, T], BF, isOutput=False)
    sin2 = nc.declare_dram_parameter("sin2", [128, T], BF, isOutput=False)
    coskv = nc.declare_dram_parameter("coskv", [128, T], BF, isOutput=False)
    sinkv = nc.declare_dram_parameter("sinkv", [128, T], BF, isOutput=False)
    r2t = nc.declare_dram_parameter("r2t", [128, 128], BF, isOutput=False)
    ident2 = nc.declare_dram_parameter("ident2", [128, 64], BF, isOutput=False)
    masks = nc.declare_dram_parameter("masks", [128, T], BF, isOutput=False)
    outt = nc.declare_dram_parameter("outt", [512, T], F32, isOutput=True)

    with _TileContext(nc) as tc:
        with (
            tc.tile_pool(name="consts", bufs=1) as consts,
            tc.tile_pool(name="big", bufs=1) as big,
            tc.tile_pool(name="wts", bufs=1) as wts,
            tc.tile_pool(name="acts", bufs=1) as acts,
            tc.tile_pool(name="work", bufs=4) as work,
            tc.tile_pool(name="exps", bufs=6) as exps,
            tc.tile_pool(name="outp", bufs=3) as outp,
            tc.tile_pool(name="psum", bufs=3, space="PSUM") as psum,
            tc.tile_pool(name="dram", bufs=1, space="DRAM") as dram,
        ):
            # ---- constants ----
            cos2_sb = consts.tile([128, T], BF)
            nc.sync.dma_start(cos2_sb[:], cos2[:])
            sin2_sb = consts.tile([128, T], BF)
            nc.sync.dma_start(sin2_sb[:], sin2[:])
            coskv_sb = consts.tile([128, T], BF)
            nc.sync.dma_start(coskv_sb[:], coskv[:])
            sinkv_sb = consts.tile([128, T], BF)
            nc.sync.dma_start(sinkv_sb[:], sinkv[:])
            masks_sb = consts.tile([128, T], BF)
            nc.sync.dma_start(masks_sb[:], masks[:])
            r2t_sb = consts.tile([128, 128], BF)
            nc.sync.dma_start(r2t_sb[:], r2t[:])
            ident2_sb = consts.tile([128, 64], BF)
            nc.sync.dma_start(ident2_sb[:], ident2[:])

            # ---- activations / weights in ----
            xt_sb = big.tile([128, 16, T], BF, tag="big")
            for fc in range(16):
                nc.sync.dma_start(xt_sb[:, fc, :], xt[fc * 128 : fc * 128 + 128, :])
            wkv_sb = wts.tile([128, 16, 128], BF)
            for fc in range(16):
                nc.sync.dma_start(
                    wkv_sb[:, fc, :], wkv[fc * 128 : fc * 128 + 128, :]
                )
            wq_sb = wts.tile([128, 16, 4, 128], BF)
            for fc in range(16):
                nc.sync.dma_start(
                    wq_sb[:, fc, :, :],
                    wq[fc * 128 : fc * 128 + 128, :].rearrange(
                        "p (qc m) -> p qc m", m=128
                    ),
                )
            wo_sb = wts.tile([128, 16, 4, 128], BF)

            # ---- kv projection + rope (k rows 0..63, v rows 64..127) ----
            kvrope_sb = acts.tile([128, T], BF)
            for tt in range(4 if STAGE >= 2 else 0):
                ts = slice(tt * 512, tt * 512 + 512)
                ps = psum.tile([128, 512], F32, tag="mm")
                for fc in range(16):
                    nc.tensor.matmul(
                        ps[:],
                        lhsT=wkv_sb[:, fc, :],
                        rhs=xt_sb[:, fc, ts],
                        start=(fc == 0),
                        stop=(fc == 15),
                    )
                kv_sb = work.tile([128, 512], BF, tag="evac")
                nc.vector.tensor_copy(kv_sb[:], ps[:])
                psu = psum.tile([128, 512], F32, tag="mm", name="psu")
                nc.tensor.matmul(
                    psu[:], lhsT=r2t_sb[:], rhs=kv_sb[:], start=True, stop=True
                )
                t1 = work.tile([128, 512], BF, tag="t1")
                nc.vector.tensor_mul(t1[:], kv_sb[:], coskv_sb[:, ts])
                t2 = work.tile([128, 512], BF, tag="t2")
                nc.vector.tensor_mul(t2[:], psu[:], sinkv_sb[:, ts])
                nc.vector.tensor_add(kvrope_sb[:, ts], t1[:], t2[:])

            # duplicate roped k into both partition halves (row-group packing)
            kdup_sb = acts.tile([128, T], BF)
            if STAGE >= 2:
                nc.sync.dma_start(kdup_sb[0:64, :], kvrope_sb[0:64, :])
                nc.sync.dma_start(kdup_sb[64:128, :], kvrope_sb[0:64, :])

            # v' chunks [128 tok, 65]: col 64 = 1.0 (softmax denominator trick)
            v1_sb = acts.tile([128, 16, 65], BF)
            nc.vector.memset(v1_sb[:, :, 64:65], 1.0)
            for kt in range(16 if STAGE >= 2 else 0):
                pst = psum.tile([128, 64], BF, tag="pav", bufs=2)
                nc.tensor.transpose(
                    pst[:],
                    kvrope_sb[64:128, kt * 128 : kt * 128 + 128],
                    ident2_sb[64:128, :],
                )
                nc.scalar.copy(v1_sb[:, kt, 0:64], pst[:])

            # ---- q projection chunks interleaved with attention head pairs ----
            qrope_sb = acts.tile([128, 4, T], BF)
            ao_q = [
                dram.tile([128, T], BF, name=f"aoq{i}") for i in range(4)
            ]
            aof_q = [
                dram.tile([512, T], BF, name=f"aofq{i}") for i in range(4)
            ]
            scale = 1.0 / np.sqrt(HD)
            aof_sb = big.tile([128, 16, T], BF, tag="big")

            for ph in range(4):

                if ph == 3 and STAGE >= 6:
                    # reload already-gathered quarters while ph3 computes
                    # (gpsimd queue is idle; these wait only for xt release)
                    for i in range(3):
                        for c in range(4):
                            nc.gpsimd.dma_start(
                                aof_sb[:, 4 * i + c, :],
                                aof_q[i][c * 128 : c * 128 + 128, :],
                            )
                if STAGE >= 3:
                  for tt in range(4):
                    ts = slice(tt * 512, tt * 512 + 512)
                    ps = psum.tile([128, 512], F32, tag="mm", name="psq")
                    for fc in range(16):
                        nc.tensor.matmul(
                            ps[:],
                            lhsT=wq_sb[:, fc, ph, :],
                            rhs=xt_sb[:, fc, ts],
                            start=(fc == 0),
                            stop=(fc == 15),
                        )
                    q_sb = work.tile([128, 512], BF, tag="evac")
                    nc.vector.tensor_copy(q_sb[:], ps[:])
                    psu = psum.tile([128, 512], F32, tag="mm", name="psu2")
                    nc.tensor.matmul(
                        psu[:], lhsT=r2t_sb[:], rhs=q_sb[:],
                        start=True, stop=True,
                    )
                    t1 = work.tile([128, 512], BF, tag="t1")
                    nc.vector.tensor_mul(t1[:], q_sb[:], cos2_sb[:, ts])
                    t2 = work.tile([128, 512], BF, tag="t2")
                    nc.vector.tensor_mul(t2[:], psu[:], sin2_sb[:, ts])
                    nc.vector.tensor_add(qrope_sb[:, ph, ts], t1[:], t2[:])

                for qb in range(4):
                    # attention for heads (2*ph, 2*ph+1), query tile qb
                    if STAGE < 4:
                        continue
                    Q0 = qb * 512
                    nkt = 4 * qb + 4
                    pav = [
                        psum.tile([65, 512], F32, tag="pav", name=f"pav{i}", bufs=2)
                        for i in range(2)
                    ]
                    for pr in range(nkt // 2):
                        kt0, kt1 = 2 * pr, 2 * pr + 1
                        # causal-active widths (tiles above the diagonal shrink)
                        j0, j1 = kt0 - 4 * qb, kt1 - 4 * qb
                        w0 = 512 if j0 < 0 else 512 - 128 * j0
                        w1 = 512 if j1 < 0 else 512 - 128 * j1
                        diag = j0 >= 0
                        # scores for both head halves interleaved so adjacent
                        # matmuls target different PE row groups (concurrent)
                        pss = [
                            psum.tile([128, 1024], F32, tag="mm", name=f"pss{i}")
                            for i in range(2)
                        ]
                        for kt, w, off in ((kt0, w0, 0), (kt1, w1, w0)):
                            for par in range(2):
                                lo, hi = (0, 64) if par == 0 else (64, 128)
                                nc.tensor.matmul(
                                    pss[par][:, off : off + w],
                                    lhsT=kdup_sb[lo:hi, kt * 128 : kt * 128 + 128],
                                    rhs=qrope_sb[lo:hi, ph, Q0 + 512 - w : Q0 + 512],
                                    start=True,
                                    stop=True,
                                )
                        e_pair = []
                        for par in range(2):
                            e_sb = exps.tile([128, 1024], BF, tag="e", name=f"e{par}")
                            nc.scalar.activation(
                                e_sb[:, 0 : w0 + w1], pss[par][:, 0 : w0 + w1],
                                AF.Exp, scale=scale,
                            )
                            if diag:
                                nc.vector.tensor_mul(
                                    e_sb[:, 0:w0], e_sb[:, 0:w0],
                                    masks_sb[:, 0:w0],
                                )
                                nc.vector.tensor_mul(
                                    e_sb[:, w0 : w0 + w1], e_sb[:, w0 : w0 + w1],
                                    masks_sb[:, 0:w1],
                                )
                            e_pair.append(e_sb)
                        for kt, w, off in ((kt0, w0, 0), (kt1, w1, w0)):
                            for par in range(2):
                                nc.tensor.matmul(
                                    pav[par][:, 512 - w : 512],
                                    lhsT=v1_sb[:, kt, :],
                                    rhs=e_pair[par][:, off : off + w],
                                    start=(kt == 0),
                                    stop=(kt == nkt - 1),
                                )
                    # evacuate unnormalized av + denominators (one copy per
                    # half), releasing the PSUM accumulators immediately; the
                    # normalization below runs off the critical path with no
                    # PE/PSUM involvement (DRAM-bounce broadcast), and both
                    # halves share one 128-lane reciprocal
                    avu = []
                    for par in range(2):
                        avu_sb = work.tile([65, 512], BF, tag="avu", name=f"avu{par}")
                        nc.scalar.copy(avu_sb[:], pav[par][:])
                        avu.append(avu_sb)
                    dden = dram.tile([2, 512], BF, tag="dden", bufs=4, name="dden")
                    for par in range(2):
                        nc.sync.dma_start(dden[par : par + 1, :], avu[par][64:65, :])
                    rden_sb = work.tile([128, 8], BF, tag="rden")
                    nc.sync.dma_start(
                        rden_sb[:],
                        bass.AP(tensor=dden.tensor, offset=dden.offset,
                                ap=[[8, 128], [1, 8]]),
                    )
                    with nc.allow_low_precision(
                        reason="bf16 softmax denominators are within tolerance"
                    ):
                        nc.vector.reciprocal(rden_sb[:], rden_sb[:])
                    rdden = dram.tile([2, 512], BF, tag="rdden", bufs=4, name="rdden")
                    nc.sync.dma_start(
                        bass.AP(tensor=rdden.tensor, offset=rdden.offset,
                                ap=[[8, 128], [1, 8]]),
                        rden_sb[:],
                    )
                    for par in range(2):
                        b_sb = work.tile([64, 512], BF, tag="bcast", name=f"b{par}")
                        nc.sync.dma_start(
                            b_sb[:],
                            bass.AP(
                                tensor=rdden.tensor,
                                offset=rdden[par : par + 1, :].offset,
                                ap=[[0, 64], [1, 512]],
                            ),
                        )
                        av_sb = work.tile([64, 512], BF, tag="av", name=f"av{par}")
                        nc.vector.tensor_mul(
                            av_sb[:], avu[par][0:64, :], b_sb[:]
                        )
                        nc.sync.dma_start(
                            ao_q[ph][64 * par : 64 * par + 64, Q0 : Q0 + 512],
                            av_sb[:],
                        )
                if STAGE >= 5:
                    # gather this head pair while later pairs compute
                    nc.gpsimd.collective_compute(
                        "AllGather",
                        mybir.AluOpType.bypass,
                        ins=[ao_q[ph].opt()],
                        outs=[aof_q[ph].opt()],
                        replica_groups=[[0, 1, 2, 3], [4, 5, 6, 7]],
                    )
                    if STAGE >= 6 and ph == 3:
                        for c in range(4):
                            nc.sync.dma_start(
                                aof_sb[:, 12 + c, :],
                                aof_q[3][c * 128 : c * 128 + 128, :],
                            )

            # wo weights land right after attention, spread across three
            # DMA-issuing queues so the 4 MB arrives in a few microseconds
            for fc in range(16 if STAGE >= 7 else 0):
                nc.sync.dma_start(
                    wo_sb[:, fc, :, :],
                    wo[fc * 128 : fc * 128 + 128, :].rearrange(
                        "p (cc m) -> p cc m", m=128
                    ),
                )

            # ---- output projection (512-column slice of wo) ----
            # aof_q[i] chunk c covers rank c, head pair i of that rank
            #   -> wo feature-chunk 4*c + i
            NWO = int(os.environ.get('KWO', '4'))
            for cc in range(NWO if STAGE >= 7 else 0):
                for tt in range(4):
                    ts = slice(tt * 512, tt * 512 + 512)
                    ps = psum.tile([128, 512], F32, tag="mm")
                    for i in range(4):
                        for c in range(4):
                            nc.tensor.matmul(
                                ps[:],
                                lhsT=wo_sb[:, 4 * c + i, cc, :],
                                rhs=aof_sb[:, 4 * i + c, ts],
                                start=(i == 0 and c == 0),
                                stop=(i == 3 and c == 3),
                            )
                    o_sb = outp.tile([128, 512], F32, tag="o")
                    nc.scalar.copy(o_sb[:], ps[:])
                    nc.sync.dma_start(outt[cc * 128 : cc * 128 + 128, ts], o_sb[:])

    return nc


def _host_tables():
    inv_freq = 1.0 / (10000.0 ** (np.arange(0, HD, 2, dtype=np.float32) / HD))
    t = np.arange(T, dtype=np.float32)
    freqs = np.einsum("i,j->ij", t, inv_freq)
    emb = np.concatenate([freqs, freqs], axis=-1)  # [T, 64]
    cosT = np.cos(emb).T.astype(np.float32)  # [64, T]
    sinT = np.sin(emb).T.astype(np.float32)

    cos2 = np.ascontiguousarray(np.vstack([cosT, cosT])).astype(BF16)
    sin2 = np.ascontiguousarray(np.vstack([sinT, sinT])).astype(BF16)
    coskv = np.ascontiguousarray(np.vstack([cosT, np.ones_like(cosT)])).astype(BF16)
    sinkv = np.ascontiguousarray(np.vstack([sinT, np.zeros_like(sinT)])).astype(BF16)

    R = np.zeros((HD, HD), dtype=np.float32)
    for d in range(32):
        R[d, d + 32] = -1.0
        R[d + 32, d] = 1.0
    r2 = np.block([[R, np.zeros_like(R)], [np.zeros_like(R), R]])
    r2t = np.ascontiguousarray(r2.T).astype(BF16)  # lhsT: matmul computes R2 @ rhs

    ident2 = np.vstack([np.eye(HD), np.eye(HD)]).astype(BF16)  # [128, 64]

    masks = np.zeros((128, T), dtype=np.float32)
    r_idx = np.arange(128)[:, None]
    c_idx = np.arange(512)[None, :]
    for j in range(4):
        masks[:, j * 512 : j * 512 + 512] = (c_idx >= 128 * j + r_idx)
    masks = masks.astype(BF16)

    return dict(
        cos2=cos2, sin2=sin2, coskv=coskv, sinkv=sinkv,
        r2t=r2t, ident2=ident2, masks=masks,
    )


_STATE = {}


def _get_nc():
    if "nc" not in _STATE:
        _STATE["nc"] = _build_nc()
        _STATE["tables"] = _host_tables()
    return _STATE["nc"], _STATE["tables"]


def kernel(x, wq, wk, wv, wo):
    nc, tables = _get_nc()

    x = np.asarray(x, dtype=np.float32)
    wq_b = np.asarray(wq, dtype=np.float32).astype(BF16)
    wo_b = np.asarray(wo, dtype=np.float32).astype(BF16)
    wk_b = np.asarray(wk, dtype=np.float32).astype(BF16)
    wv_b = np.asarray(wv, dtype=np.float32).astype(BF16)

    in_maps = []
    xt_b = [np.ascontiguousarray(x[b].T).astype(BF16) for b in range(2)]
    for core in range(N_CORES):
        b, g = core // 4, core % 4
        m = dict(tables)
        m["xt"] = xt_b[b]
        m["wq"] = np.ascontiguousarray(wq_b[:, 512 * g : 512 * g + 512])
        m["wkv"] = np.ascontiguousarray(
            np.concatenate(
                [wk_b[:, 64 * g : 64 * g + 64], wv_b[:, 64 * g : 64 * g + 64]],
                axis=1,
            )
        )
        m["wo"] = np.ascontiguousarray(wo_b[:, 512 * g : 512 * g + 512])
        in_maps.append(m)

    res = run_bass_kernel_spmd(
        nc, in_maps, core_ids=list(range(N_CORES)), trace=False
    )

    out = np.empty((2, T, DIM), dtype=np.float32)
    for core in range(N_CORES):
        b, g = core // 4, core % 4
        out[b][:, 512 * g : 512 * g + 512] = res.results[core]["outt"].T
    return out


# revision 37
# speedup vs baseline: 1.2004x; 1.0541x over previous
"""Distributed GQA attention block (dense_transformer) for 8 TRN2 NeuronCores.

Reference computation (all fp32):
    q = (x @ wq)  -> RoPE;  k = (x @ wk) -> RoPE;  v = x @ wv
    causal softmax(q k^T / sqrt(64)) @ v  (GQA: 32 q heads, 4 kv heads)
    out = attn_out @ wo

Sharding: core (b, g) for b in {0,1}, g in {0..3} handles batch b, q-heads
8g..8g+7, kv-head g (data-parallel over batch x tensor-parallel over GQA
groups).  Each core computes attn_outT for its heads ([512, 2048],
feature-major), AllGathers within its 4-core batch group, and applies a
512-column slice of wo.  Outputs are disjoint -> host concat only.

All activations/weights are kept feature-major (transposed) on chip so every
matmul contracts over the partition dim with no on-chip transposes except a
single small one for v.  Matmul compute in bf16 (fp32 PSUM accumulate).
"""

import json

import numpy as np
import ml_dtypes

import concourse.bass as bass
import concourse.bass2jax as bass2jax
import concourse.mybir as mybir
import concourse.tile as tile
from concourse.tile import VectorClock, ScopedClock
from concourse.bass_utils import compile_bir_kernel, run_bass_kernel_spmd

_MAX_WAITS = 1  # this walrus build rejects instructions with more sem waits


def _split_excess_waits(bir_json, max_waits=_MAX_WAITS):
    """Hoist excess per-instruction sem waits onto injected same-engine NoOps.

    The TRN2 ISA encoding in this neuronxcc build allows at most `max_waits`
    sync-wait commands per instruction; Tile's sem assigner can emit more.
    A NoOp inserted immediately before the instruction on the same engine is
    semantically identical (the engine blocks at the same program point).
    """
    d = json.loads(bir_json)
    changed = False
    for fn in d.get("functions", []):
        for bb in fn.get("blocks", []):
            insts = bb.get("instructions", [])
            new = []
            for ins in insts:
                si = ins.get("sync_info")
                waits = (si or {}).get("on_wait") or []
                if len(waits) > max_waits:
                    changed = True
                    excess, keep = waits[:-max_waits], waits[-max_waits:]
                    for i in range(0, len(excess), max_waits):
                        new.append(
                            {
                                "debug": ins.get("debug", 0),
                                "engine": ins["engine"],
                                "ins": [],
                                "name": f"{ins['name']}-wsplit{i}",
                                "opcode": "NoOp",
                                "outs": [],
                                "sync_info": {
                                    "on_update": [],
                                    "on_wait": excess[i : i + max_waits],
                                },
                            }
                        )
                    si["on_wait"] = keep
                new.append(ins)
            bb["instructions"] = new
    if not changed:
        return bir_json
    return json.dumps(d).encode()


def _patched_compile_bir_kernel(bir_json, tmpdir, neff_name="file.neff"):
    return compile_bir_kernel(_split_excess_waits(bir_json), tmpdir, neff_name)


bass2jax.compile_bir_kernel = _patched_compile_bir_kernel

BF16 = ml_dtypes.bfloat16
F32 = mybir.dt.float32
BF = mybir.dt.bfloat16

DIM = 2048
T = 2048
HD = 64
N_CORES = 8
AF = mybir.ActivationFunctionType


class _TileContext(tile.TileContext):
    """TileContext whose final drain carries one sem wait per instruction.

    The walrus build in this image rejects a Drain carrying several sync
    waits ("Too many sync wait commands"), so emit individual single-wait
    NOPs on the sync engine first, then an unadorned drain + barriers.
    """

    def _drain_and_barrier(self, tick_clock, wait_clock):
        gc = tick_clock.global_clock
        vals = eval(repr(gc).replace("VectorClock(", "").rstrip(")"))
        for i, v in enumerate(vals):
            if v:
                single = [0] * len(vals)
                single[i] = v
                nop = self.nc.sync.nop(nofuse=True)
                wait_clock.add_sem_waits(
                    nop.ins, ScopedClock({None: VectorClock(single)})
                )
        self.nc.sync.drain()
        self.nc.all_engine_barrier()
        popped = self.nc._tile_sem_poison_stack.pop()
        assert popped is self._sem_poison
        self.nc.clear_and_free_semaphores(list(self.sems.allocated().values()))
        self.nc.all_engine_barrier()


def _build_nc():
    import os
    STAGE = int(os.environ.get("KSTAGE", "9"))
    nc = bass.Bass("TRN2")

    xt = nc.declare_dram_parameter("xt", [DIM, T], BF, isOutput=False)
    wq = nc.declare_dram_parameter("wq", [DIM, 512], BF, isOutput=False)
    wkv = nc.declare_dram_parameter("wkv", [DIM, 128], BF, isOutput=False)
    wo = nc.declare_dram_parameter("wo", [DIM, 512], BF, isOutput=False)
    cos2 = nc.declare_dram_parameter("cos2", [128, T], BF, isOutput=False)
    sin2 = nc.declare_dram_parameter("sin2", [128, T], BF, isOutput=False)
    coskv = nc.declare_dram_parameter("coskv", [128, T], BF, isOutput=False)
    sinkv = nc.declare_dram_parameter("sinkv", [128, T], BF, isOutput=False)
    r2t = nc.declare_dram_parameter("r2t", [128, 128], BF, isOutput=False)
    ident2 = nc.declare_dram_parameter("ident2", [128, 64], BF, isOutput=False)
    masks = nc.declare_dram_parameter("masks", [128, T], BF, isOutput=False)
    outt = nc.declare_dram_parameter("outt", [512, T], F32, isOutput=True)

    with _TileContext(nc) as tc:
        with (
            tc.tile_pool(name="consts", bufs=1) as consts,
            tc.tile_pool(name="big", bufs=1) as big,
            tc.tile_pool(name="wts", bufs=1) as wts,
            tc.tile_pool(name="acts", bufs=1) as acts,
            tc.tile_pool(name="work", bufs=4) as work,
            tc.tile_pool(name="exps", bufs=6) as exps,
            tc.tile_pool(name="outp", bufs=3) as outp,
            tc.tile_pool(name="psum", bufs=3, space="PSUM") as psum,
            tc.tile_pool(name="dram", bufs=1, space="DRAM") as dram,
        ):
            # ---- constants ----
            cos2_sb = consts.tile([128, T], BF)
            nc.sync.dma_start(cos2_sb[:], cos2[:])
            sin2_sb = consts.tile([128, T], BF)
            nc.sync.dma_start(sin2_sb[:], sin2[:])
            coskv_sb = consts.tile([128, T], BF)
            nc.sync.dma_start(coskv_sb[:], coskv[:])
            sinkv_sb = consts.tile([128, T], BF)
            nc.sync.dma_start(sinkv_sb[:], sinkv[:])
            masks_sb = consts.tile([128, T], BF)
            nc.sync.dma_start(masks_sb[:], masks[:])
            r2t_sb = consts.tile([128, 128], BF)
            nc.sync.dma_start(r2t_sb[:], r2t[:])
            ident2_sb = consts.tile([128, 64], BF)
            nc.sync.dma_start(ident2_sb[:], ident2[:])

            # ---- activations / weights in ----
            xt_sb = big.tile([128, 16, T], BF, tag="big")
            for fc in range(16):
                nc.sync.dma_start(xt_sb[:, fc, :], xt[fc * 128 : fc * 128 + 128, :])
            wkv_sb = wts.tile([128, 16, 128], BF)
            for fc in range(16):
                nc.sync.dma_start(
                    wkv_sb[:, fc, :], wkv[fc * 128 : fc * 128 + 128, :]
                )
            wq_sb = wts.tile([128, 16, 4, 128], BF)
            for fc in range(16):
                nc.sync.dma_start(
                    wq_sb[:, fc, :, :],
                    wq[fc * 128 : fc * 128 + 128, :].rearrange(
                        "p (qc m) -> p qc m", m=128
                    ),
                )
            wo_sb = wts.tile([128, 16, 4, 128], BF)

            # ---- kv projection + rope (k rows 0..63, v rows 64..127) ----
            kvrope_sb = acts.tile([128, T], BF)
            for tt in range(4 if STAGE >= 2 else 0):
                ts = slice(tt * 512, tt * 512 + 512)
                ps = psum.tile([128, 512], F32, tag="mm")
                for fc in range(16):
                    nc.tensor.matmul(
                        ps[:],
                        lhsT=wkv_sb[:, fc, :],
                        rhs=xt_sb[:, fc, ts],
                        start=(fc == 0),
                        stop=(fc == 15),
                    )
                kv_sb = work.tile([128, 512], BF, tag="evac")
                nc.vector.tensor_copy(kv_sb[:], ps[:])
                psu = psum.tile([128, 512], F32, tag="mm", name="psu")
                nc.tensor.matmul(
                    psu[:], lhsT=r2t_sb[:], rhs=kv_sb[:], start=True, stop=True
                )
                t1 = work.tile([128, 512], BF, tag="t1")
                nc.vector.tensor_mul(t1[:], kv_sb[:], coskv_sb[:, ts])
                t2 = work.tile([128, 512], BF, tag="t2")
                nc.vector.tensor_mul(t2[:], psu[:], sinkv_sb[:, ts])
                nc.vector.tensor_add(kvrope_sb[:, ts], t1[:], t2[:])

            # duplicate roped k into both partition halves (row-group packing)
            kdup_sb = acts.tile([128, T], BF)
            if STAGE >= 2:
                nc.sync.dma_start(kdup_sb[0:64, :], kvrope_sb[0:64, :])
                nc.sync.dma_start(kdup_sb[64:128, :], kvrope_sb[0:64, :])

            # v' chunks [128 tok, 65]: col 64 = 1.0 (softmax denominator trick)
            v1_sb = acts.tile([128, 16, 65], BF)
            nc.vector.memset(v1_sb[:, :, 64:65], 1.0)
            for kt in range(16 if STAGE >= 2 else 0):
                pst = psum.tile([128, 64], BF, tag="pav", bufs=2)
                nc.tensor.transpose(
                    pst[:],
                    kvrope_sb[64:128, kt * 128 : kt * 128 + 128],
                    ident2_sb[64:128, :],
                )
                nc.scalar.copy(v1_sb[:, kt, 0:64], pst[:])

            # ---- q projection chunks interleaved with attention head pairs ----
            qrope_sb = acts.tile([128, 4, T], BF)
            ao_q = [
                dram.tile([128, T], BF, name=f"aoq{i}") for i in range(4)
            ]
            aof_q = [
                dram.tile([512, T], BF, name=f"aofq{i}") for i in range(4)
            ]
            scale = 1.0 / np.sqrt(HD)
            aof_sb = big.tile([128, 16, T], BF, tag="big")

            for ph in range(4):

                if ph == 3 and STAGE >= 6:
                    # reload already-gathered quarters while ph3 computes
                    # (gpsimd queue is idle; these wait only for xt release)
                    for i in range(3):
                        for c in range(4):
                            nc.gpsimd.dma_start(
                                aof_sb[:, 4 * i + c, :],
                                aof_q[i][c * 128 : c * 128 + 128, :],
                            )
                if STAGE >= 3:
                  for tt in range(4):
                    ts = slice(tt * 512, tt * 512 + 512)
                    ps = psum.tile([128, 512], F32, tag="mm", name="psq")
                    for fc in range(16):
                        nc.tensor.matmul(
                            ps[:],
                            lhsT=wq_sb[:, fc, ph, :],
                            rhs=xt_sb[:, fc, ts],
                            start=(fc == 0),
                            stop=(fc == 15),
                        )
                    q_sb = work.tile([128, 512], BF, tag="evac")
                    nc.vector.tensor_copy(q_sb[:], ps[:])
                    psu = psum.tile([128, 512], F32, tag="mm", name="psu2")
                    nc.tensor.matmul(
                        psu[:], lhsT=r2t_sb[:], rhs=q_sb[:],
                        start=True, stop=True,
                    )
                    t1 = work.tile([128, 512], BF, tag="t1")
                    nc.vector.tensor_mul(t1[:], q_sb[:], cos2_sb[:, ts])
                    t2 = work.tile([128, 512], BF, tag="t2")
                    nc.vector.tensor_mul(t2[:], psu[:], sin2_sb[:, ts])
                    nc.vector.tensor_add(qrope_sb[:, ph, ts], t1[:], t2[:])

                for qb in range(4):
                    # attention for heads (2*ph, 2*ph+1), query tile qb
                    if STAGE < 4:
                        continue
                    Q0 = qb * 512
                    nkt = 4 * qb + 4
                    pav = [
                        psum.tile([65, 512], F32, tag="pav", name=f"pav{i}", bufs=2)
                        for i in range(2)
                    ]
                    for pr in range(nkt // 2):
                        kt0, kt1 = 2 * pr, 2 * pr + 1
                        # causal-active widths (tiles above the diagonal shrink)
                        j0, j1 = kt0 - 4 * qb, kt1 - 4 * qb
                        w0 = 512 if j0 < 0 else 512 - 128 * j0
                        w1 = 512 if j1 < 0 else 512 - 128 * j1
                        diag = j0 >= 0
                        # scores for both head halves interleaved so adjacent
                        # matmuls target different PE row groups (concurrent)
                        pss = [
                            psum.tile([128, 1024], F32, tag="mm", name=f"pss{i}")
                            for i in range(2)
                        ]
                        for kt, w, off in ((kt0, w0, 0), (kt1, w1, w0)):
                            for par in range(2):
                                lo, hi = (0, 64) if par == 0 else (64, 128)
                                nc.tensor.matmul(
                                    pss[par][:, off : off + w],
                                    lhsT=kdup_sb[lo:hi, kt * 128 : kt * 128 + 128],
                                    rhs=qrope_sb[lo:hi, ph, Q0 + 512 - w : Q0 + 512],
                                    start=True,
                                    stop=True,
                                )
                        e_pair = []
                        for par in range(2):
                            e_sb = exps.tile([128, 1024], BF, tag="e", name=f"e{par}")
                            nc.scalar.activation(
                                e_sb[:, 0 : w0 + w1], pss[par][:, 0 : w0 + w1],
                                AF.Exp, scale=scale,
                            )
                            if diag:
                                nc.vector.tensor_mul(
                                    e_sb[:, 0:w0], e_sb[:, 0:w0],
                                    masks_sb[:, 0:w0],
                                )
                                nc.vector.tensor_mul(
                                    e_sb[:, w0 : w0 + w1], e_sb[:, w0 : w0 + w1],
                                    masks_sb[:, 0:w1],
                                )
                            e_pair.append(e_sb)
                        for kt, w, off in ((kt0, w0, 0), (kt1, w1, w0)):
                            for par in range(2):
                                nc.tensor.matmul(
                                    pav[par][:, 512 - w : 512],
                                    lhsT=v1_sb[:, kt, :],
                                    rhs=e_pair[par][:, off : off + w],
                                    start=(kt == 0),
                                    stop=(kt == nkt - 1),
                                )
                    # evacuate unnormalized av + denominators (one copy per
                    # half), releasing the PSUM accumulators immediately; the
                    # normalization below runs off the critical path with no
                    # PE/PSUM involvement (DRAM-bounce broadcast), and both
                    # halves share one 128-lane reciprocal
                    avu = []
                    for par in range(2):
                        avu_sb = work.tile([65, 512], BF, tag="avu", name=f"avu{par}")
                        nc.scalar.copy(avu_sb[:], pav[par][:])
                        avu.append(avu_sb)
                    dden = dram.tile([2, 512], BF, tag="dden", bufs=4, name="dden")
                    for par in range(2):
                        nc.gpsimd.dma_start(dden[par : par + 1, :], avu[par][64:65, :])
                    rden_sb = work.tile([128, 8], BF, tag="rden")
                    nc.gpsimd.dma_start(
                        rden_sb[:],
                        bass.AP(tensor=dden.tensor, offset=dden.offset,
                                ap=[[8, 128], [1, 8]]),
                    )
                    with nc.allow_low_precision(
                        reason="bf16 softmax denominators are within tolerance"
                    ):
                        nc.vector.reciprocal(rden_sb[:], rden_sb[:])
                    rdden = dram.tile([2, 512], BF, tag="rdden", bufs=4, name="rdden")
                    nc.gpsimd.dma_start(
                        bass.AP(tensor=rdden.tensor, offset=rdden.offset,
                                ap=[[8, 128], [1, 8]]),
                        rden_sb[:],
                    )
                    for par in range(2):
                        b_sb = work.tile([64, 512], BF, tag="bcast", name=f"b{par}")
                        nc.gpsimd.dma_start(
                            b_sb[:],
                            bass.AP(
                                tensor=rdden.tensor,
                                offset=rdden[par : par + 1, :].offset,
                                ap=[[0, 64], [1, 512]],
                            ),
                        )
                        av_sb = work.tile([64, 512], BF, tag="av", name=f"av{par}")
                        nc.vector.tensor_mul(
                            av_sb[:], avu[par][0:64, :], b_sb[:]
                        )
                        nc.sync.dma_start(
                            ao_q[ph][64 * par : 64 * par + 64, Q0 : Q0 + 512],
                            av_sb[:],
                        )
                if STAGE >= 5:
                    # gather this head pair while later pairs compute
                    nc.gpsimd.collective_compute(
                        "AllGather",
                        mybir.AluOpType.bypass,
                        ins=[ao_q[ph].opt()],
                        outs=[aof_q[ph].opt()],
                        replica_groups=[[0, 1, 2, 3], [4, 5, 6, 7]],
                    )
                    if STAGE >= 6 and ph == 3:
                        for c in range(4):
                            nc.sync.dma_start(
                                aof_sb[:, 12 + c, :],
                                aof_q[3][c * 128 : c * 128 + 128, :],
                            )

            # wo weights land right after attention, spread across three
            # DMA-issuing queues so the 4 MB arrives in a few microseconds
            for fc in range(16 if STAGE >= 7 else 0):
                nc.sync.dma_start(
                    wo_sb[:, fc, :, :],
                    wo[fc * 128 : fc * 128 + 128, :].rearrange(
                        "p (cc m) -> p cc m", m=128
                    ),
                )

            # ---- output projection (512-column slice of wo) ----
            # aof_q[i] chunk c covers rank c, head pair i of that rank
            #   -> wo feature-chunk 4*c + i
            NWO = int(os.environ.get('KWO', '4'))
            for cc in range(NWO if STAGE >= 7 else 0):
                for tt in range(4):
                    ts = slice(tt * 512, tt * 512 + 512)
                    ps = psum.tile([128, 512], F32, tag="mm")
                    for i in range(4):
                        for c in range(4):
                            nc.tensor.matmul(
                                ps[:],
                                lhsT=wo_sb[:, 4 * c + i, cc, :],
                                rhs=aof_sb[:, 4 * i + c, ts],
                                start=(i == 0 and c == 0),
                                stop=(i == 3 and c == 3),
                            )
                    o_sb = outp.tile([128, 512], F32, tag="o")
                    nc.scalar.copy(o_sb[:], ps[:])
                    nc.sync.dma_start(outt[cc * 128 : cc * 128 + 128, ts], o_sb[:])

    return nc


def _host_tables():
    inv_freq = 1.0 / (10000.0 ** (np.arange(0, HD, 2, dtype=np.float32) / HD))
    t = np.arange(T, dtype=np.float32)
    freqs = np.einsum("i,j->ij", t, inv_freq)
    emb = np.concatenate([freqs, freqs], axis=-1)  # [T, 64]
    cosT = np.cos(emb).T.astype(np.float32)  # [64, T]
    sinT = np.sin(emb).T.astype(np.float32)

    cos2 = np.ascontiguousarray(np.vstack([cosT, cosT])).astype(BF16)
    sin2 = np.ascontiguousarray(np.vstack([sinT, sinT])).astype(BF16)
    coskv = np.ascontiguousarray(np.vstack([cosT, np.ones_like(cosT)])).astype(BF16)
    sinkv = np.ascontiguousarray(np.vstack([sinT, np.zeros_like(sinT)])).astype(BF16)

    R = np.zeros((HD, HD), dtype=np.float32)
    for d in range(32):
        R[d, d + 32] = -1.0
        R[d + 32, d] = 1.0
    r2 = np.block([[R, np.zeros_like(R)], [np.zeros_like(R), R]])
    r2t = np.ascontiguousarray(r2.T).astype(BF16)  # lhsT: matmul computes R2 @ rhs

    ident2 = np.vstack([np.eye(HD), np.eye(HD)]).astype(BF16)  # [128, 64]

    masks = np.zeros((128, T), dtype=np.float32)
    r_idx = np.arange(128)[:, None]
    c_idx = np.arange(512)[None, :]
    for j in range(4):
        masks[:, j * 512 : j * 512 + 512] = (c_idx >= 128 * j + r_idx)
    masks = masks.astype(BF16)

    return dict(
        cos2=cos2, sin2=sin2, coskv=coskv, sinkv=sinkv,
        r2t=r2t, ident2=ident2, masks=masks,
    )


_STATE = {}


def _get_nc():
    if "nc" not in _STATE:
        _STATE["nc"] = _build_nc()
        _STATE["tables"] = _host_tables()
    return _STATE["nc"], _STATE["tables"]


def kernel(x, wq, wk, wv, wo):
    nc, tables = _get_nc()

    x = np.asarray(x, dtype=np.float32)
    wq_b = np.asarray(wq, dtype=np.float32).astype(BF16)
    wo_b = np.asarray(wo, dtype=np.float32).astype(BF16)
    wk_b = np.asarray(wk, dtype=np.float32).astype(BF16)
    wv_b = np.asarray(wv, dtype=np.float32).astype(BF16)

    in_maps = []
    xt_b = [np.ascontiguousarray(x[b].T).astype(BF16) for b in range(2)]
    for core in range(N_CORES):
        b, g = core // 4, core % 4
        m = dict(tables)
        m["xt"] = xt_b[b]
        m["wq"] = np.ascontiguousarray(wq_b[:, 512 * g : 512 * g + 512])
        m["wkv"] = np.ascontiguousarray(
            np.concatenate(
                [wk_b[:, 64 * g : 64 * g + 64], wv_b[:, 64 * g : 64 * g + 64]],
                axis=1,
            )
        )
        m["wo"] = np.ascontiguousarray(wo_b[:, 512 * g : 512 * g + 512])
        in_maps.append(m)

    res = run_bass_kernel_spmd(
        nc, in_maps, core_ids=list(range(N_CORES)), trace=False
    )

    out = np.empty((2, T, DIM), dtype=np.float32)
    for core in range(N_CORES):
        b, g = core // 4, core % 4
        out[b][:, 512 * g : 512 * g + 512] = res.results[core]["outt"].T
    return out


# revision 38
# speedup vs baseline: 1.2030x; 1.0021x over previous
"""Distributed GQA attention block (dense_transformer) for 8 TRN2 NeuronCores.

Reference computation (all fp32):
    q = (x @ wq)  -> RoPE;  k = (x @ wk) -> RoPE;  v = x @ wv
    causal softmax(q k^T / sqrt(64)) @ v  (GQA: 32 q heads, 4 kv heads)
    out = attn_out @ wo

Sharding: core (b, g) for b in {0,1}, g in {0..3} handles batch b, q-heads
8g..8g+7, kv-head g (data-parallel over batch x tensor-parallel over GQA
groups).  Each core computes attn_outT for its heads ([512, 2048],
feature-major), AllGathers within its 4-core batch group, and applies a
512-column slice of wo.  Outputs are disjoint -> host concat only.

All activations/weights are kept feature-major (transposed) on chip so every
matmul contracts over the partition dim with no on-chip transposes except a
single small one for v.  Matmul compute in bf16 (fp32 PSUM accumulate).
"""

import json

import numpy as np
import ml_dtypes

import concourse.bass as bass
import concourse.bass2jax as bass2jax
import concourse.mybir as mybir
import concourse.tile as tile
from concourse.tile import VectorClock, ScopedClock
from concourse.bass_utils import compile_bir_kernel, run_bass_kernel_spmd

_MAX_WAITS = 1  # this walrus build rejects instructions with more sem waits


def _split_excess_waits(bir_json, max_waits=_MAX_WAITS):
    """Hoist excess per-instruction sem waits onto injected same-engine NoOps.

    The TRN2 ISA encoding in this neuronxcc build allows at most `max_waits`
    sync-wait commands per instruction; Tile's sem assigner can emit more.
    A NoOp inserted immediately before the instruction on the same engine is
    semantically identical (the engine blocks at the same program point).
    """
    d = json.loads(bir_json)
    changed = False
    for fn in d.get("functions", []):
        for bb in fn.get("blocks", []):
            insts = bb.get("instructions", [])
            new = []
            for ins in insts:
                si = ins.get("sync_info")
                waits = (si or {}).get("on_wait") or []
                if len(waits) > max_waits:
                    changed = True
                    excess, keep = waits[:-max_waits], waits[-max_waits:]
                    for i in range(0, len(excess), max_waits):
                        new.append(
                            {
                                "debug": ins.get("debug", 0),
                                "engine": ins["engine"],
                                "ins": [],
                                "name": f"{ins['name']}-wsplit{i}",
                                "opcode": "NoOp",
                                "outs": [],
                                "sync_info": {
                                    "on_update": [],
                                    "on_wait": excess[i : i + max_waits],
                                },
                            }
                        )
                    si["on_wait"] = keep
                new.append(ins)
            bb["instructions"] = new
    if not changed:
        return bir_json
    return json.dumps(d).encode()


def _patched_compile_bir_kernel(bir_json, tmpdir, neff_name="file.neff"):
    return compile_bir_kernel(_split_excess_waits(bir_json), tmpdir, neff_name)


bass2jax.compile_bir_kernel = _patched_compile_bir_kernel

BF16 = ml_dtypes.bfloat16
F32 = mybir.dt.float32
BF = mybir.dt.bfloat16

DIM = 2048
T = 2048
HD = 64
N_CORES = 8
AF = mybir.ActivationFunctionType


class _TileContext(tile.TileContext):
    """TileContext whose final drain carries one sem wait per instruction.

    The walrus build in this image rejects a Drain carrying several sync
    waits ("Too many sync wait commands"), so emit individual single-wait
    NOPs on the sync engine first, then an unadorned drain + barriers.
    """

    def _drain_and_barrier(self, tick_clock, wait_clock):
        gc = tick_clock.global_clock
        vals = eval(repr(gc).replace("VectorClock(", "").rstrip(")"))
        for i, v in enumerate(vals):
            if v:
                single = [0] * len(vals)
                single[i] = v
                nop = self.nc.sync.nop(nofuse=True)
                wait_clock.add_sem_waits(
                    nop.ins, ScopedClock({None: VectorClock(single)})
                )
        self.nc.sync.drain()
        self.nc.all_engine_barrier()
        popped = self.nc._tile_sem_poison_stack.pop()
        assert popped is self._sem_poison
        self.nc.clear_and_free_semaphores(list(self.sems.allocated().values()))
        self.nc.all_engine_barrier()


def _build_nc():
    import os
    STAGE = int(os.environ.get("KSTAGE", "9"))
    nc = bass.Bass("TRN2")

    xt = nc.declare_dram_parameter("xt", [DIM, T], BF, isOutput=False)
    wq = nc.declare_dram_parameter("wq", [DIM, 512], BF, isOutput=False)
    wkv = nc.declare_dram_parameter("wkv", [DIM, 128], BF, isOutput=False)
    wo = nc.declare_dram_parameter("wo", [DIM, 512], BF, isOutput=False)
    cos2 = nc.declare_dram_parameter("cos2", [128, T], BF, isOutput=False)
    sin2 = nc.declare_dram_parameter("sin2", [128, T], BF, isOutput=False)
    coskv = nc.declare_dram_parameter("coskv", [128, T], BF, isOutput=False)
    sinkv = nc.declare_dram_parameter("sinkv", [128, T], BF, isOutput=False)
    r2t = nc.declare_dram_parameter("r2t", [128, 128], BF, isOutput=False)
    ident2 = nc.declare_dram_parameter("ident2", [128, 64], BF, isOutput=False)
    masks = nc.declare_dram_parameter("masks", [128, T], BF, isOutput=False)
    outt = nc.declare_dram_parameter("outt", [512, T], F32, isOutput=True)

    with _TileContext(nc) as tc:
        with (
            tc.tile_pool(name="consts", bufs=1) as consts,
            tc.tile_pool(name="big", bufs=1) as big,
            tc.tile_pool(name="wts", bufs=1) as wts,
            tc.tile_pool(name="acts", bufs=1) as acts,
            tc.tile_pool(name="work", bufs=4) as work,
            tc.tile_pool(name="exps", bufs=6) as exps,
            tc.tile_pool(name="outp", bufs=3) as outp,
            tc.tile_pool(name="psum", bufs=3, space="PSUM") as psum,
            tc.tile_pool(name="dram", bufs=1, space="DRAM") as dram,
        ):
            # ---- constants (r2t first: it feeds the PE warm-up burst) ----
            r2t_sb = consts.tile([128, 128], BF)
            nc.sync.dma_start(r2t_sb[:], r2t[:])

            # PE warm-up: ~5us of back-to-back matmuls during the DMA intro
            # lifts the HAM clock gate to 2.4 GHz before real compute starts
            pwarm = psum.tile([128, 512], F32, tag="mm", name="pwarm")
            for wi in range(40):
                nc.tensor.matmul(
                    pwarm[:, 0:128], lhsT=r2t_sb[:], rhs=r2t_sb[:],
                    start=True, stop=True,
                )

            # ---- activations / weights in ----
            xt_sb = big.tile([128, 16, T], BF, tag="big")
            for fc in range(16):
                nc.sync.dma_start(xt_sb[:, fc, :], xt[fc * 128 : fc * 128 + 128, :])

            cos2_sb = consts.tile([128, T], BF)
            nc.sync.dma_start(cos2_sb[:], cos2[:])
            sin2_sb = consts.tile([128, T], BF)
            nc.sync.dma_start(sin2_sb[:], sin2[:])
            coskv_sb = consts.tile([128, T], BF)
            nc.sync.dma_start(coskv_sb[:], coskv[:])
            sinkv_sb = consts.tile([128, T], BF)
            nc.sync.dma_start(sinkv_sb[:], sinkv[:])
            masks_sb = consts.tile([128, T], BF)
            nc.sync.dma_start(masks_sb[:], masks[:])
            ident2_sb = consts.tile([128, 64], BF)
            nc.sync.dma_start(ident2_sb[:], ident2[:])
            wkv_sb = wts.tile([128, 16, 128], BF)
            for fc in range(16):
                nc.sync.dma_start(
                    wkv_sb[:, fc, :], wkv[fc * 128 : fc * 128 + 128, :]
                )
            wq_sb = wts.tile([128, 16, 4, 128], BF)
            for fc in range(16):
                nc.sync.dma_start(
                    wq_sb[:, fc, :, :],
                    wq[fc * 128 : fc * 128 + 128, :].rearrange(
                        "p (qc m) -> p qc m", m=128
                    ),
                )
            wo_sb = wts.tile([128, 16, 4, 128], BF)

            # ---- kv projection + rope (k rows 0..63, v rows 64..127) ----
            kvrope_sb = acts.tile([128, T], BF)
            for tt in range(4 if STAGE >= 2 else 0):
                ts = slice(tt * 512, tt * 512 + 512)
                ps = psum.tile([128, 512], F32, tag="mm")
                for fc in range(16):
                    nc.tensor.matmul(
                        ps[:],
                        lhsT=wkv_sb[:, fc, :],
                        rhs=xt_sb[:, fc, ts],
                        start=(fc == 0),
                        stop=(fc == 15),
                    )
                kv_sb = work.tile([128, 512], BF, tag="evac")
                nc.vector.tensor_copy(kv_sb[:], ps[:])
                psu = psum.tile([128, 512], F32, tag="mm", name="psu")
                nc.tensor.matmul(
                    psu[:], lhsT=r2t_sb[:], rhs=kv_sb[:], start=True, stop=True
                )
                t1 = work.tile([128, 512], BF, tag="t1")
                nc.vector.tensor_mul(t1[:], kv_sb[:], coskv_sb[:, ts])
                t2 = work.tile([128, 512], BF, tag="t2")
                nc.vector.tensor_mul(t2[:], psu[:], sinkv_sb[:, ts])
                nc.vector.tensor_add(kvrope_sb[:, ts], t1[:], t2[:])

            # duplicate roped k into both partition halves (row-group packing)
            kdup_sb = acts.tile([128, T], BF)
            if STAGE >= 2:
                nc.sync.dma_start(kdup_sb[0:64, :], kvrope_sb[0:64, :])
                nc.sync.dma_start(kdup_sb[64:128, :], kvrope_sb[0:64, :])

            # v' chunks [128 tok, 65]: col 64 = 1.0 (softmax denominator trick)
            v1_sb = acts.tile([128, 16, 65], BF)
            nc.vector.memset(v1_sb[:, :, 64:65], 1.0)
            for kt in range(16 if STAGE >= 2 else 0):
                pst = psum.tile([128, 64], BF, tag="pav", bufs=2)
                nc.tensor.transpose(
                    pst[:],
                    kvrope_sb[64:128, kt * 128 : kt * 128 + 128],
                    ident2_sb[64:128, :],
                )
                nc.scalar.copy(v1_sb[:, kt, 0:64], pst[:])

            # ---- q projection chunks interleaved with attention head pairs ----
            qrope_sb = acts.tile([128, 4, T], BF)
            ao_q = [
                dram.tile([128, T], BF, name=f"aoq{i}") for i in range(4)
            ]
            aof_q = [
                dram.tile([512, T], BF, name=f"aofq{i}") for i in range(4)
            ]
            scale = 1.0 / np.sqrt(HD)
            aof_sb = big.tile([128, 16, T], BF, tag="big")

            for ph in range(4):

                if ph == 3 and STAGE >= 6:
                    # reload already-gathered quarters while ph3 computes
                    # (gpsimd queue is idle; these wait only for xt release)
                    for i in range(3):
                        for c in range(4):
                            nc.gpsimd.dma_start(
                                aof_sb[:, 4 * i + c, :],
                                aof_q[i][c * 128 : c * 128 + 128, :],
                            )
                if STAGE >= 3:
                  for tt in range(4):
                    ts = slice(tt * 512, tt * 512 + 512)
                    ps = psum.tile([128, 512], F32, tag="mm", name="psq")
                    for fc in range(16):
                        nc.tensor.matmul(
                            ps[:],
                            lhsT=wq_sb[:, fc, ph, :],
                            rhs=xt_sb[:, fc, ts],
                            start=(fc == 0),
                            stop=(fc == 15),
                        )
                    q_sb = work.tile([128, 512], BF, tag="evac")
                    nc.vector.tensor_copy(q_sb[:], ps[:])
                    psu = psum.tile([128, 512], F32, tag="mm", name="psu2")
                    nc.tensor.matmul(
                        psu[:], lhsT=r2t_sb[:], rhs=q_sb[:],
                        start=True, stop=True,
                    )
                    t1 = work.tile([128, 512], BF, tag="t1")
                    nc.vector.tensor_mul(t1[:], q_sb[:], cos2_sb[:, ts])
                    t2 = work.tile([128, 512], BF, tag="t2")
                    nc.vector.tensor_mul(t2[:], psu[:], sin2_sb[:, ts])
                    nc.vector.tensor_add(qrope_sb[:, ph, ts], t1[:], t2[:])

                for qb in range(4):
                    # attention for heads (2*ph, 2*ph+1), query tile qb
                    if STAGE < 4:
                        continue
                    Q0 = qb * 512
                    nkt = 4 * qb + 4
                    pav = [
                        psum.tile([65, 512], F32, tag="pav", name=f"pav{i}", bufs=2)
                        for i in range(2)
                    ]
                    for pr in range(nkt // 2):
                        kt0, kt1 = 2 * pr, 2 * pr + 1
                        # causal-active widths (tiles above the diagonal shrink)
                        j0, j1 = kt0 - 4 * qb, kt1 - 4 * qb
                        w0 = 512 if j0 < 0 else 512 - 128 * j0
                        w1 = 512 if j1 < 0 else 512 - 128 * j1
                        diag = j0 >= 0
                        # scores for both head halves interleaved so adjacent
                        # matmuls target different PE row groups (concurrent)
                        pss = [
                            psum.tile([128, 1024], F32, tag="mm", name=f"pss{i}")
                            for i in range(2)
                        ]
                        for kt, w, off in ((kt0, w0, 0), (kt1, w1, w0)):
                            for par in range(2):
                                lo, hi = (0, 64) if par == 0 else (64, 128)
                                nc.tensor.matmul(
                                    pss[par][:, off : off + w],
                                    lhsT=kdup_sb[lo:hi, kt * 128 : kt * 128 + 128],
                                    rhs=qrope_sb[lo:hi, ph, Q0 + 512 - w : Q0 + 512],
                                    start=True,
                                    stop=True,
                                )
                        e_pair = []
                        for par in range(2):
                            e_sb = exps.tile([128, 1024], BF, tag="e", name=f"e{par}")
                            nc.scalar.activation(
                                e_sb[:, 0 : w0 + w1], pss[par][:, 0 : w0 + w1],
                                AF.Exp, scale=scale,
                            )
                            if diag:
                                nc.vector.tensor_mul(
                                    e_sb[:, 0:w0], e_sb[:, 0:w0],
                                    masks_sb[:, 0:w0],
                                )
                                nc.vector.tensor_mul(
                                    e_sb[:, w0 : w0 + w1], e_sb[:, w0 : w0 + w1],
                                    masks_sb[:, 0:w1],
                                )
                            e_pair.append(e_sb)
                        for kt, w, off in ((kt0, w0, 0), (kt1, w1, w0)):
                            for par in range(2):
                                nc.tensor.matmul(
                                    pav[par][:, 512 - w : 512],
                                    lhsT=v1_sb[:, kt, :],
                                    rhs=e_pair[par][:, off : off + w],
                                    start=(kt == 0),
                                    stop=(kt == nkt - 1),
                                )
                    # evacuate unnormalized av + denominators (one copy per
                    # half), releasing the PSUM accumulators immediately; the
                    # normalization below runs off the critical path with no
                    # PE/PSUM involvement (DRAM-bounce broadcast), and both
                    # halves share one 128-lane reciprocal
                    avu = []
                    for par in range(2):
                        avu_sb = work.tile([65, 512], BF, tag="avu", name=f"avu{par}")
                        nc.scalar.copy(avu_sb[:], pav[par][:])
                        avu.append(avu_sb)
                    dden = dram.tile([2, 512], BF, tag="dden", bufs=4, name="dden")
                    for par in range(2):
                        nc.gpsimd.dma_start(dden[par : par + 1, :], avu[par][64:65, :])
                    rden_sb = work.tile([128, 8], BF, tag="rden")
                    nc.gpsimd.dma_start(
                        rden_sb[:],
                        bass.AP(tensor=dden.tensor, offset=dden.offset,
                                ap=[[8, 128], [1, 8]]),
                    )
                    with nc.allow_low_precision(
                        reason="bf16 softmax denominators are within tolerance"
                    ):
                        nc.vector.reciprocal(rden_sb[:], rden_sb[:])
                    rdden = dram.tile([2, 512], BF, tag="rdden", bufs=4, name="rdden")
                    nc.gpsimd.dma_start(
                        bass.AP(tensor=rdden.tensor, offset=rdden.offset,
                                ap=[[8, 128], [1, 8]]),
                        rden_sb[:],
                    )
                    for par in range(2):
                        b_sb = work.tile([64, 512], BF, tag="bcast", name=f"b{par}")
                        nc.gpsimd.dma_start(
                            b_sb[:],
                            bass.AP(
                                tensor=rdden.tensor,
                                offset=rdden[par : par + 1, :].offset,
                                ap=[[0, 64], [1, 512]],
                            ),
                        )
                        av_sb = work.tile([64, 512], BF, tag="av", name=f"av{par}")
                        nc.vector.tensor_mul(
                            av_sb[:], avu[par][0:64, :], b_sb[:]
                        )
                        nc.sync.dma_start(
                            ao_q[ph][64 * par : 64 * par + 64, Q0 : Q0 + 512],
                            av_sb[:],
                        )
                if STAGE >= 5:
                    # gather this head pair while later pairs compute
                    nc.gpsimd.collective_compute(
                        "AllGather",
                        mybir.AluOpType.bypass,
                        ins=[ao_q[ph].opt()],
                        outs=[aof_q[ph].opt()],
                        replica_groups=[[0, 1, 2, 3], [4, 5, 6, 7]],
                    )
                    if STAGE >= 6 and ph == 3:
                        for c in range(4):
                            nc.sync.dma_start(
                                aof_sb[:, 12 + c, :],
                                aof_q[3][c * 128 : c * 128 + 128, :],
                            )

            # wo weights land right after attention, spread across three
            # DMA-issuing queues so the 4 MB arrives in a few microseconds
            for fc in range(16 if STAGE >= 7 else 0):
                nc.sync.dma_start(
                    wo_sb[:, fc, :, :],
                    wo[fc * 128 : fc * 128 + 128, :].rearrange(
                        "p (cc m) -> p cc m", m=128
                    ),
                )

            # ---- output projection (512-column slice of wo) ----
            # aof_q[i] chunk c covers rank c, head pair i of that rank
            #   -> wo feature-chunk 4*c + i
            NWO = int(os.environ.get('KWO', '4'))
            for cc in range(NWO if STAGE >= 7 else 0):
                for tt in range(4):
                    ts = slice(tt * 512, tt * 512 + 512)
                    ps = psum.tile([128, 512], F32, tag="mm")
                    for i in range(4):
                        for c in range(4):
                            nc.tensor.matmul(
                                ps[:],
                                lhsT=wo_sb[:, 4 * c + i, cc, :],
                                rhs=aof_sb[:, 4 * i + c, ts],
                                start=(i == 0 and c == 0),
                                stop=(i == 3 and c == 3),
                            )
                    o_sb = outp.tile([128, 512], F32, tag="o")
                    nc.scalar.copy(o_sb[:], ps[:])
                    nc.sync.dma_start(outt[cc * 128 : cc * 128 + 128, ts], o_sb[:])

    return nc


def _host_tables():
    inv_freq = 1.0 / (10000.0 ** (np.arange(0, HD, 2, dtype=np.float32) / HD))
    t = np.arange(T, dtype=np.float32)
    freqs = np.einsum("i,j->ij", t, inv_freq)
    emb = np.concatenate([freqs, freqs], axis=-1)  # [T, 64]
    cosT = np.cos(emb).T.astype(np.float32)  # [64, T]
    sinT = np.sin(emb).T.astype(np.float32)

    cos2 = np.ascontiguousarray(np.vstack([cosT, cosT])).astype(BF16)
    sin2 = np.ascontiguousarray(np.vstack([sinT, sinT])).astype(BF16)
    coskv = np.ascontiguousarray(np.vstack([cosT, np.ones_like(cosT)])).astype(BF16)
    sinkv = np.ascontiguousarray(np.vstack([sinT, np.zeros_like(sinT)])).astype(BF16)

    R = np.zeros((HD, HD), dtype=np.float32)
    for d in range(32):
        R[d, d + 32] = -1.0
        R[d + 32, d] = 1.0
    r2 = np.block([[R, np.zeros_like(R)], [np.zeros_like(R), R]])
    r2t = np.ascontiguousarray(r2.T).astype(BF16)  # lhsT: matmul computes R2 @ rhs

    ident2 = np.vstack([np.eye(HD), np.eye(HD)]).astype(BF16)  # [128, 64]

    masks = np.zeros((128, T), dtype=np.float32)
    r_idx = np.arange(128)[:, None]
    c_idx = np.arange(512)[None, :]
    for j in range(4):
        masks[:, j * 512 : j * 512 + 512] = (c_idx >= 128 * j + r_idx)
    masks = masks.astype(BF16)

    return dict(
        cos2=cos2, sin2=sin2, coskv=coskv, sinkv=sinkv,
        r2t=r2t, ident2=ident2, masks=masks,
    )


_STATE = {}


def _get_nc():
    if "nc" not in _STATE:
        _STATE["nc"] = _build_nc()
        _STATE["tables"] = _host_tables()
    return _STATE["nc"], _STATE["tables"]


def kernel(x, wq, wk, wv, wo):
    nc, tables = _get_nc()

    x = np.asarray(x, dtype=np.float32)
    wq_b = np.asarray(wq, dtype=np.float32).astype(BF16)
    wo_b = np.asarray(wo, dtype=np.float32).astype(BF16)
    wk_b = np.asarray(wk, dtype=np.float32).astype(BF16)
    wv_b = np.asarray(wv, dtype=np.float32).astype(BF16)

    in_maps = []
    xt_b = [np.ascontiguousarray(x[b].T).astype(BF16) for b in range(2)]
    for core in range(N_CORES):
        b, g = core // 4, core % 4
        m = dict(tables)
        m["xt"] = xt_b[b]
        m["wq"] = np.ascontiguousarray(wq_b[:, 512 * g : 512 * g + 512])
        m["wkv"] = np.ascontiguousarray(
            np.concatenate(
                [wk_b[:, 64 * g : 64 * g + 64], wv_b[:, 64 * g : 64 * g + 64]],
                axis=1,
            )
        )
        m["wo"] = np.ascontiguousarray(wo_b[:, 512 * g : 512 * g + 512])
        in_maps.append(m)

    res = run_bass_kernel_spmd(
        nc, in_maps, core_ids=list(range(N_CORES)), trace=False
    )

    out = np.empty((2, T, DIM), dtype=np.float32)
    for core in range(N_CORES):
        b, g = core // 4, core % 4
        out[b][:, 512 * g : 512 * g + 512] = res.results[core]["outt"].T
    return out
